# revision 1
# baseline (speedup 1.0000x reference)
"""Bass/Trainium2 kernel for nn_CasualSelfAttention (B=4, T=2048, D=1024, H=16, dk=64).

Sharding: batch (4) x head-group (2) = 8 cores. Each core computes 8 heads of one
batch element end-to-end (QKV projections, attention, WO partial product); the
host sums the two head-group partials per batch and folds the free-dim biases.

All big matmuls run in float32r (full PE rate, ~tf32 precision). Softmax sums are
obtained by augmenting V with a ones column (M=65 PV matmuls); normalization uses
a DVE reciprocal plus a K=1 PE broadcast matmul.
"""
import sys
import os

sys.path.insert(0, '/opt/trn_rl_repo')

import numpy as np
import ml_dtypes
import orjson

import concourse.bass as bass
import concourse.tile as tile
import concourse.mybir as mybir
from concourse.bass_utils import run_bass_kernel_spmd

# ---------------------------------------------------------------- waitsplit
# The walrus build in this container accepts at most ONE semaphore wait per
# engine instruction.  Tile emits multi-wait sync_info; split the extras into
# single-wait NoOps on the same engine stream (in-order => semantically equal).
_ws_counter = [0]


_SELF_WAIT_ENGINES = ("Activation", "DVE")


def _split_instruction_waits(inst, out_list):
    si = inst.get("sync_info")
    if not si or not si.get("on_wait"):
        out_list.append(inst)
        return
    waits = si["on_wait"]
    # ACT/DVE execute strictly in order, so a compute instruction's wait on
    # its OWN engine's semaphore (slot-reuse WAW vs an older instruction on
    # the same engine) is always already satisfied — drop it instead of
    # spending a NoOp dispatch on the bottleneck ACT stream.
    eng = inst.get("engine")
    if (eng in _SELF_WAIT_ENGINES
            and inst.get("opcode") not in ("Drain", "EventSemaphore", "NoOp")):
        kept = [w for w in waits
                if w.get("ant_name", "").rsplit("_", 1)[0] != eng]
        if kept != waits:
            si = dict(si)
            si["on_wait"] = kept
            inst = dict(inst)
            inst["sync_info"] = si
            waits = kept
    if len(waits) <= 1:
        out_list.append(inst)
        return
    for w in waits[:-1]:
        _ws_counter[0] += 1
        out_list.append({
            "debug": inst.get("debug", 0),
            "engine": inst.get("engine"),
            "ins": [],
            "name": f"I-wsplit-{_ws_counter[0]}",
            "opcode": "NoOp",
            "outs": [],
            "sync_info": {"on_update": [], "on_wait": [w]},
        })
    si = dict(si)
    si["on_wait"] = [waits[-1]]
    inst = dict(inst)
    inst["sync_info"] = si
    out_list.append(inst)


def fix_multiwait_json(bir_bytes):
    d = orjson.loads(bir_bytes)
    for fn in d["functions"]:
        for bb in fn["blocks"]:
            new = []
            for inst in bb["instructions"]:
                _split_instruction_waits(inst, new)
            bb["instructions"] = new
    return orjson.dumps(d)


class WaitSplitBass(bass.Bass):
    def to_json_bytes(self):
        return fix_multiwait_json(super().to_json_bytes())


# ---------------------------------------------------------------- kernel build
P = 128
B, T, D = 4, 2048, 1024
NH_LOC = 8            # heads per core
NP = NH_LOC // 2      # head pairs per core
DK = 64
DC = D // P           # 8 d_model chunks
SC = T // P           # 16 s-chunks
NTB = T // 512        # 4 t-blocks
f32 = mybir.dt.float32
f32r = mybir.dt.float32r
bf16 = mybir.dt.bfloat16
AF = mybir.ActivationFunctionType
MULT = mybir.AluOpType.mult

_nc_cache = [None]


def build_nc():
    if _nc_cache[0] is not None:
        return _nc_cache[0]
    nc = WaitSplitBass()
    xq_t = nc.dram_tensor("xq_t", [D, T], f32r, kind="ExternalInput")
    xk_t = nc.dram_tensor("xk_t", [D, T], f32r, kind="ExternalInput")
    xv_t = nc.dram_tensor("xv_t", [D, T], f32r, kind="ExternalInput")
    wq_t = nc.dram_tensor("wq_t", [D, 512], f32r, kind="ExternalInput")
    wk_t = nc.dram_tensor("wk_t", [D, 512], f32r, kind="ExternalInput")
    wv_t = nc.dram_tensor("wv_t", [D, 512], f32r, kind="ExternalInput")
    wo_t = nc.dram_tensor("wo_t", [512, D], f32r, kind="ExternalInput")
    bq_d = nc.dram_tensor("bq_d", [P, NP], f32, kind="ExternalInput")
    bk_d = nc.dram_tensor("bk_d", [P, NP], f32, kind="ExternalInput")
    ones_d = nc.dram_tensor("ones_d", [1, 64], f32r, kind="ExternalInput")
    out = nc.dram_tensor("out", [T, D], f32, kind="ExternalOutput")

    with tile.TileContext(nc) as tc:
        with tc.tile_pool(name="persist", bufs=1) as persist, \
             tc.tile_pool(name="psProj", bufs=2, space="PSUM") as psProj, \
             tc.tile_pool(name="psS", bufs=2, space="PSUM") as psS, \
             tc.tile_pool(name="psPV", bufs=1, space="PSUM") as psPV:

            # ---- persistent tiles ----
            qT2 = [persist.tile([P, T], f32r, tag=f"qT2_{p}", name=f"qT2_{p}")
                   for p in range(NP)]
            kT2 = [persist.tile([P, T], f32r, tag=f"kT2_{p}", name=f"kT2_{p}")
                   for p in range(NP)]
            V_aug = persist.tile([P, SC, NH_LOC, 65], f32r, name="V_aug")
            nc.vector.memset(V_aug[:, :, :, 64].bitcast(f32), 1.0)
            bq_s = persist.tile([P, NP], f32, name="bq_s")
            nc.sync.dma_start(bq_s[:], bq_d[:])
            bk_s = persist.tile([P, NP], f32, name="bk_s")
            nc.sync.dma_start(bk_s[:], bk_d[:])
            ones64 = persist.tile([1, 64], f32r, name="ones64")
            nc.sync.dma_start(ones64[:], ones_d[:])

            # ---- phase A: projections (weights + X^T streamed per chunk) ----
            # 4 concurrent psum groups: 2 slots borrowed from the (idle)
            # scores pool + 2 from psProj.
            def alloc4(stem):
                ps = [psS.tile([P, 1024], f32, tag="scores",
                               name=f"{stem}_s{j}")[:, 0:512] for j in range(2)]
                ps += [psProj.tile([P, 512], f32, tag="proj",
                                   name=f"{stem}_p{j}") for j in range(2)]
                return ps

            ctx_00 = persist.tile([P, 512], f32r, name="ctx_00")
            with tc.tile_pool(name="pearly", bufs=2) as pearly, \
                 tc.tile_pool(name="wpool", bufs=1) as wpool, \
                 tc.tile_pool(name="xpool", bufs=8) as xpool:
                wq = wpool.tile([P, DC, 512], f32r, tag="wq", name="wq")
                wk = wpool.tile([P, DC, 512], f32r, tag="wk", name="wk")
                wv = wpool.tile([P, DC, 512], f32r, tag="wv", name="wv")
                for c in range(DC):
                    nc.sync.dma_start(wq[:, c], wq_t[c * P:(c + 1) * P, :])
                    nc.sync.dma_start(wk[:, c], wk_t[c * P:(c + 1) * P, :])
                    nc.sync.dma_start(wv[:, c], wv_t[c * P:(c + 1) * P, :])

                for tb in range(NTB):
                    ts_ = slice(tb * 512, (tb + 1) * 512)
                    # q then k: 4 pair-groups, chunk-outer accumulation
                    for qk, (w_t, xdram, b_s, dst) in enumerate(
                            ((wq, xq_t, bq_s, qT2), (wk, xk_t, bk_s, kT2))):
                        ps4 = alloc4(f"psqk{tb}_{qk}")
                        for c in range(DC):
                            x_c = xpool.tile([P, 512], f32r, tag=f"x{qk}",
                                             name=f"x{qk}_{tb}_{c}")
                            nc.sync.dma_start(x_c[:], xdram[c * P:(c + 1) * P, ts_])
                            for p in range(NP):
                                nc.tensor.matmul(
                                    ps4[p][:], w_t[:, c, p * P:(p + 1) * P],
                                    x_c[:], start=(c == 0), stop=(c == DC - 1))
                        for p in range(NP):
                            nc.vector.tensor_scalar_add(
                                dst[p][:, ts_], ps4[p][:], b_s[:, p:p + 1])
                    # v: 4 t-tile groups, chunk-outer
                    ps4 = alloc4(f"psv{tb}")
                    for c in range(DC):
                        xv_c = xpool.tile([P, 512], f32r, tag="xv",
                                          name=f"xv_{tb}_{c}")
                        nc.sync.dma_start(xv_c[:], xv_t[c * P:(c + 1) * P, ts_])
                        for ti in range(4):
                            nc.tensor.matmul(
                                ps4[ti][:], xv_c[:, ti * P:(ti + 1) * P],
                                wv[:, c], start=(c == 0), stop=(c == DC - 1))
                    for ti in range(4):
                        tt = tb * 4 + ti
                        nc.vector.tensor_copy(
                            V_aug[:, tt, :, 0:64],
                            ps4[ti][:].rearrange("p (h d) -> p h d", d=64))

                    # early attention for (t-block 0, pair 0): its PV psum is
                    # idle during phase A and every dep of s-chunk quarter tb
                    # is produced by A(tb) — run it here so ACT starts ~100us
                    # earlier instead of idling through all projections.
                    if tb == 0:
                        pv_e0 = psPV.tile([65, 512], f32, tag="pv0",
                                          name="pv0_0_0")
                        pv_e1 = psPV.tile([65, 512], f32, tag="pv1",
                                          name="pv1_0_0")
                    for sc in range(4 * tb, 4 * tb + 4):
                        ss = slice(sc * P, (sc + 1) * P)
                        ps_s = psS.tile([P, 1024], f32, tag="scores",
                                        name=f"ps_s_0_0_{sc}")
                        nc.tensor.matmul(
                            ps_s[:, 0:512], qT2[0][0:64, ss],
                            kT2[0][0:64, 0:512], start=True, stop=True,
                            tile_position=(0, 0))
                        nc.tensor.matmul(
                            ps_s[:, 512:1024], qT2[0][64:128, ss],
                            kT2[0][64:128, 0:512], start=True, stop=True,
                            tile_position=(64, 0))
                        p_t = pearly.tile([P, 1024], f32r, tag="pe",
                                          name=f"pe_{sc}")
                        nc.scalar.activation(p_t[:], ps_s[:], AF.Exp,
                                             scale=0.125)
                        nc.tensor.matmul(
                            pv_e0[:], V_aug[:, sc, 0, :], p_t[:, 0:512],
                            start=(sc == 0), stop=(sc == SC - 1))
                        nc.tensor.matmul(
                            pv_e1[:], V_aug[:, sc, 1, :], p_t[:, 512:1024],
                            start=(sc == 0), stop=(sc == SC - 1))

            # ---- phases B+C per t-block ----
            with tc.tile_pool(name="ppool", bufs=5) as ppool, \
                 tc.tile_pool(name="rbpool", bufs=2) as rbpool, \
                 tc.tile_pool(name="ctxpool", bufs=2) as ctxpool, \
                 tc.tile_pool(name="wopool", bufs=1) as wopool, \
                 tc.tile_pool(name="opool", bufs=3) as opool:
                    wo = wopool.tile([P, NP, D], f32r, name="wo")
                    for p in range(NP):
                        nc.sync.dma_start(wo[:, p], wo_t[p * P:(p + 1) * P, :])

                    def flush_evac(pend):
                        # normalize pair into its ctx tile:
                        # ctx[h] = pv[h][0:64] * bcast(1 / pv[h][64])
                        tb, p, pv0, pv1, ctx_p = pend
                        for h, pv in ((0, pv0), (1, pv1)):
                            r_t = rbpool.tile([1, 512], f32r, tag="r",
                                              name=f"r_{tb}_{p}_{h}")
                            with nc.allow_low_precision(reason="softmax recip"):
                                nc.vector.reciprocal(r_t[:], pv[64:65, :])
                            ps_rb = psProj.tile([64, 512], f32, tag="proj",
                                                name=f"ps_rb_{tb}_{p}_{h}")
                            nc.tensor.matmul(ps_rb[:], ones64[:], r_t[:],
                                             start=True, stop=True)
                            rb_s = rbpool.tile([64, 512], f32, tag="rb",
                                               name=f"rb_{tb}_{p}_{h}")
                            nc.vector.tensor_copy(rb_s[:], ps_rb[:])
                            nc.vector.tensor_tensor(
                                ctx_p[h * 64:(h + 1) * 64, :],
                                pv[0:64, :], rb_s[:], MULT)

                    def emit_wo_chunk(wtb, wctx, ti, ob):
                        # one [128t, 512o] WO output tile of t-block wtb
                        ps_o = psProj.tile([P, 512], f32, tag="proj",
                                           name=f"ps_o_{wtb}_{ti}_{ob}")
                        for p in range(NP):
                            nc.tensor.matmul(
                                ps_o[:], wctx[p][:, ti * P:(ti + 1) * P],
                                wo[:, p, ob * 512:(ob + 1) * 512],
                                start=(p == 0), stop=(p == NP - 1))
                        o_t = opool.tile([P, 512], f32, tag="o",
                                         name=f"o_{wtb}_{ti}_{ob}")
                        nc.vector.tensor_copy(o_t[:], ps_o[:])
                        nc.sync.dma_start(
                            out[wtb * 512 + ti * P: wtb * 512 + (ti + 1) * P,
                                ob * 512:(ob + 1) * 512], o_t[:])

                    # (t-block 0, pair 0) already ran during phase A; seed its
                    # deferred evacuation so pair 1's sc==2 flush handles it.
                    pending = (0, 0, pv_e0, pv_e1, ctx_00)
                    pending_wo = None     # (tb, ctx_tb) whose WO is deferred
                    for tb in range(NTB):
                        ts_ = slice(tb * 512, (tb + 1) * 512)
                        ctx_tb = [ctx_00] if tb == 0 else []
                        for p in range(NP):
                            if tb == 0 and p == 0:
                                continue
                            # -- B: attention for (pair p, t-block tb) --
                            pv0 = psPV.tile([65, 512], f32, tag="pv0",
                                            name=f"pv0_{tb}_{p}")
                            pv1 = psPV.tile([65, 512], f32, tag="pv1",
                                            name=f"pv1_{tb}_{p}")
                            # Defer the previous pair's PV-psum evacuation (and
                            # the previous t-block's WO chunks) past this
                            # pair's first score/exp groups so ACT stays fed
                            # while PE runs the evac/WO work in its slack.
                            stash = []
                            for sc in range(SC):
                                ss = slice(sc * P, (sc + 1) * P)
                                ps_s = psS.tile([P, 1024], f32, tag="scores",
                                                name=f"ps_s_{tb}_{p}_{sc}")
                                nc.tensor.matmul(
                                    ps_s[:, 0:512], qT2[p][0:64, ss],
                                    kT2[p][0:64, ts_], start=True, stop=True,
                                    tile_position=(0, 0))
                                nc.tensor.matmul(
                                    ps_s[:, 512:1024], qT2[p][64:128, ss],
                                    kT2[p][64:128, ts_], start=True, stop=True,
                                    tile_position=(64, 0))
                                p_t = ppool.tile([P, 1024], f32r, tag="p",
                                                 name=f"p_{tb}_{p}_{sc}")
                                nc.scalar.activation(p_t[:], ps_s[:], AF.Exp,
                                                     scale=0.125)
                                if pending is not None and sc < 2:
                                    stash.append((sc, p_t))
                                    continue
                                if pending is not None and sc == 2:
                                    flush_evac(pending)
                                    pending = None
                                for s0, pt0 in stash:
                                    nc.tensor.matmul(
                                        pv0[:], V_aug[:, s0, 2 * p, :],
                                        pt0[:, 0:512],
                                        start=(s0 == 0), stop=False)
                                    nc.tensor.matmul(
                                        pv1[:], V_aug[:, s0, 2 * p + 1, :],
                                        pt0[:, 512:1024],
                                        start=(s0 == 0), stop=False)
                                stash = []
                                nc.tensor.matmul(
                                    pv0[:], V_aug[:, sc, 2 * p, :],
                                    p_t[:, 0:512],
                                    start=(sc == 0), stop=(sc == SC - 1))
                                nc.tensor.matmul(
                                    pv1[:], V_aug[:, sc, 2 * p + 1, :],
                                    p_t[:, 512:1024],
                                    start=(sc == 0), stop=(sc == SC - 1))
                                # sprinkle the previous t-block's 8 WO
                                # chunks across pairs 0-1, every other sc,
                                # to stay under the ACT rate per slot
                                if (pending_wo is not None and p <= 1
                                        and 2 <= sc <= 9 and (sc % 2) == 0):
                                    widx = p * 4 + (sc - 2) // 2
                                    emit_wo_chunk(pending_wo[0], pending_wo[1],
                                                  widx // 2, widx % 2)
                                    if widx == 7:
                                        pending_wo = None
                            ctx_p = ctxpool.tile([P, 512], f32r, tag=f"ctx{p}",
                                                 name=f"ctx_{tb}_{p}")
                            pending = (tb, p, pv0, pv1, ctx_p)
                            ctx_tb.append(ctx_p)
                        pending_wo = (tb, ctx_tb)

                    # tail: last pair's evac + last t-block's WO
                    if pending is not None:
                        flush_evac(pending)
                        pending = None
                    if pending_wo is not None:
                        for ti in range(4):
                            for ob in range(2):
                                emit_wo_chunk(pending_wo[0], pending_wo[1],
                                              ti, ob)
                        pending_wo = None
    _nc_cache[0] = nc
    return nc


# ---------------------------------------------------------------- host side
def make_in_maps(keys, queries, values, WK_w, WK_b, WQ_w, WQ_b, WV_w, WV_b, WO_w):
    keys = np.asarray(keys, dtype=np.float32)
    queries = np.asarray(queries, dtype=np.float32)
    values = np.asarray(values, dtype=np.float32)
    xq_b = [np.ascontiguousarray(queries[b].T) for b in range(B)]
    xk_b = [np.ascontiguousarray(keys[b].T) for b in range(B)]
    xv_b = [np.ascontiguousarray(values[b].T) for b in range(B)]
    ones = np.ones((1, 64), np.float32)
    in_maps = []
    for c in range(8):
        b, g = c // 2, c % 2
        sl = slice(512 * g, 512 * (g + 1))
        in_maps.append({
            "xq_t": xq_b[b], "xk_t": xk_b[b], "xv_t": xv_b[b],
            "wq_t": np.ascontiguousarray(np.asarray(WQ_w, np.float32)[sl, :].T),
            "wk_t": np.ascontiguousarray(np.asarray(WK_w, np.float32)[sl, :].T),
            "wv_t": np.ascontiguousarray(np.asarray(WV_w, np.float32)[sl, :].T),
            "wo_t": np.ascontiguousarray(np.asarray(WO_w, np.float32)[:, sl].T),
            "bq_d": np.ascontiguousarray(
                np.asarray(WQ_b, np.float32)[sl].reshape(NP, P).T),
            "bk_d": np.ascontiguousarray(
                np.asarray(WK_b, np.float32)[sl].reshape(NP, P).T),
            "ones_d": ones,
        })
    return in_maps


def kernel(keys, queries, values, pad_mask, WK_w, WK_b, WQ_w, WQ_b, WV_w, WV_b,
           WO_w, WO_b):
    nc = build_nc()
    in_maps = make_in_maps(keys, queries, values, WK_w, WK_b, WQ_w, WQ_b,
                           WV_w, WV_b, WO_w)
    res = run_bass_kernel_spmd(nc, in_maps, list(range(8)))
    # free-dim biases folded on host: WO_b directly; WV_b exactly via
    # sum_g (WV_b_g @ WO_g^T) = WV_b @ WO_w^T  (attention rows sum to 1).
    bias = (np.asarray(WO_b, np.float64)
            + np.asarray(WV_b, np.float64) @ np.asarray(WO_w, np.float64).T)
    out = np.empty((B, T, D), np.float32)
    for b in range(B):
        out[b] = (res.results[2 * b]["out"].astype(np.float64)
                  + res.results[2 * b + 1]["out"].astype(np.float64)
                  + bias).astype(np.float32)
    return out



# revision 2
# speedup vs baseline: 8.2070x; 8.2070x over previous
"""Bass/Trainium2 kernel for nn_CasualSelfAttention (B=4, T=2048, D=1024, H=16, dk=64).

Sharding: pure batch-parallel over 4 cores; each core computes all 16 heads of
one batch element end-to-end (QKV projections, attention, WO). No partial sums
across cores -> each core's output is exact (host folds the free-dim biases).

Inputs are packed into two fp16 blobs to minimize per-call transfer cost:
  xblob [3072, 2048]  (per-core):  rows 0:1024 xq^T, 1024:2048 xk^T, 2048:3072 xv^T
  wblob [2176, 2048]  (same for all cores): rows 0:1024 = [wq^T | wk^T],
        rows 1024:2048 = [wv^T | wo^T], rows 2048:2176 cols 0:16 = [bq | bk]

All big matmuls run in fp16 operands (fp32 PSUM accumulation -> only operand
quantization error, ~2^-11). Softmax sums use an augmented-V ones column
(M=65 PV matmuls); normalization uses a DVE reciprocal plus a K=1 PE
broadcast matmul.
"""
import sys
import os

sys.path.insert(0, '/opt/trn_rl_repo')

import numpy as np
import orjson

import concourse.bass as bass
import concourse.tile as tile
import concourse.mybir as mybir
from concourse.bass_utils import run_bass_kernel_spmd

# ---------------------------------------------------------------- waitsplit
# The walrus build in this container accepts at most ONE semaphore wait per
# engine instruction.  Tile emits multi-wait sync_info; split the extras into
# single-wait NoOps on the same engine stream (in-order => semantically equal).
_ws_counter = [0]


_SELF_WAIT_ENGINES = ("Activation", "DVE")


def _split_instruction_waits(inst, out_list):
    si = inst.get("sync_info")
    if not si or not si.get("on_wait"):
        out_list.append(inst)
        return
    waits = si["on_wait"]
    # ACT/DVE execute strictly in order, so a compute instruction's wait on
    # its OWN engine's semaphore (slot-reuse WAW vs an older instruction on
    # the same engine) is always already satisfied — drop it instead of
    # spending a NoOp dispatch on the bottleneck ACT stream.
    eng = inst.get("engine")
    if (eng in _SELF_WAIT_ENGINES
            and inst.get("opcode") not in ("Drain", "EventSemaphore", "NoOp")):
        kept = [w for w in waits
                if w.get("ant_name", "").rsplit("_", 1)[0] != eng]
        if kept != waits:
            si = dict(si)
            si["on_wait"] = kept
            inst = dict(inst)
            inst["sync_info"] = si
            waits = kept
    if len(waits) <= 1:
        out_list.append(inst)
        return
    for w in waits[:-1]:
        _ws_counter[0] += 1
        out_list.append({
            "debug": inst.get("debug", 0),
            "engine": inst.get("engine"),
            "ins": [],
            "name": f"I-wsplit-{_ws_counter[0]}",
            "opcode": "NoOp",
            "outs": [],
            "sync_info": {"on_update": [], "on_wait": [w]},
        })
    si = dict(si)
    si["on_wait"] = [waits[-1]]
    inst = dict(inst)
    inst["sync_info"] = si
    out_list.append(inst)


def fix_multiwait_json(bir_bytes):
    d = orjson.loads(bir_bytes)
    for fn in d["functions"]:
        for bb in fn["blocks"]:
            new = []
            for inst in bb["instructions"]:
                _split_instruction_waits(inst, new)
            bb["instructions"] = new
    return orjson.dumps(d)


class WaitSplitBass(bass.Bass):
    def to_json_bytes(self):
        return fix_multiwait_json(super().to_json_bytes())


# ---------------------------------------------------------------- kernel build
P = 128
B, T, D = 4, 2048, 1024
N_CORES = 4           # batch-parallel
NH_LOC = 16           # heads per core (all of them)
NP = NH_LOC // 2      # head pairs per core = 8
DK = 64
DC = D // P           # 8 d_model chunks
SC = T // P           # 16 s-chunks
NTB = T // 512        # 4 t-blocks
f32 = mybir.dt.float32
f32r = mybir.dt.float32r
fp16 = mybir.dt.float16
AF = mybir.ActivationFunctionType
MULT = mybir.AluOpType.mult

# xblob row offsets
XQ_OFF, XK_OFF, XV_OFF = 0, 1024, 2048
# wblob row offsets
WQK_OFF = 0       # cols 0:1024 wq^T, 1024:2048 wk^T
WVO_OFF = 1024    # cols 0:1024 wv^T, 1024:2048 wo^T
BIAS_OFF = 2048   # rows 2048:2176, cols 0:8 bq, 8:16 bk

_nc_cache = [None]


def build_nc():
    if _nc_cache[0] is not None:
        return _nc_cache[0]
    nc = WaitSplitBass()
    xblob = nc.dram_tensor("xblob", [3072, 2048], fp16, kind="ExternalInput")
    wblob = nc.dram_tensor("wblob", [2176, 2048], fp16, kind="ExternalInput")
    out = nc.dram_tensor("out", [T, D], fp16, kind="ExternalOutput")

    with tile.TileContext(nc) as tc:
        with tc.tile_pool(name="persist", bufs=1) as persist, \
             tc.tile_pool(name="psProj", bufs=2, space="PSUM") as psProj, \
             tc.tile_pool(name="psS", bufs=2, space="PSUM") as psS, \
             tc.tile_pool(name="psPV", bufs=1, space="PSUM") as psPV:

            # ---- persistent tiles ----
            qT2 = [persist.tile([P, T], fp16, tag=f"qT2_{p}", name=f"qT2_{p}")
                   for p in range(NP)]
            kT2 = [persist.tile([P, T], fp16, tag=f"kT2_{p}", name=f"kT2_{p}")
                   for p in range(NP)]
            V_aug = persist.tile([P, SC, NH_LOC, 65], fp16, name="V_aug")
            nc.vector.memset(V_aug[:, :, :, 64], 1.0)
            b16 = persist.tile([P, 16], fp16, name="b16")
            nc.sync.dma_start(b16[:], wblob[BIAS_OFF:BIAS_OFF + P, 0:16])
            bqk = persist.tile([P, 16], f32, name="bqk")
            nc.vector.tensor_copy(bqk[:], b16[:])
            ones64 = persist.tile([1, 64], f32r, name="ones64")
            nc.vector.memset(ones64[:].bitcast(f32), 1.0)

            # ---- phase A: projections (weights + X^T streamed per chunk) ----
            # 4 concurrent psum groups: 2 slots borrowed from the (idle)
            # scores pool + 2 from psProj.  8 head-pairs (or 2 v-halves) are
            # processed in 2 waves of 4 groups.
            def alloc4(stem):
                ps = [psS.tile([P, 1024], f32, tag="scores",
                               name=f"{stem}_s{j}")[:, 0:512] for j in range(2)]
                ps += [psProj.tile([P, 512], f32, tag="proj",
                                   name=f"{stem}_p{j}") for j in range(2)]
                return ps

            ctx_00 = persist.tile([P, 512], fp16, name="ctx_00")
            with tc.tile_pool(name="pearly", bufs=2) as pearly, \
                 tc.tile_pool(name="wpool", bufs=1) as wpool, \
                 tc.tile_pool(name="xpool", bufs=8) as xpool:
                wq = wpool.tile([P, DC, 1024], fp16, tag="wq", name="wq")
                wk = wpool.tile([P, DC, 1024], fp16, tag="wk", name="wk")
                wv = wpool.tile([P, DC, 1024], fp16, tag="wv", name="wv")
                for c in range(DC):
                    nc.sync.dma_start(
                        wq[:, c], wblob[WQK_OFF + c * P:WQK_OFF + (c + 1) * P,
                                        0:1024])
                    nc.sync.dma_start(
                        wk[:, c], wblob[WQK_OFF + c * P:WQK_OFF + (c + 1) * P,
                                        1024:2048])
                    nc.sync.dma_start(
                        wv[:, c], wblob[WVO_OFF + c * P:WVO_OFF + (c + 1) * P,
                                        0:1024])

                for tb in range(NTB):
                    ts_ = slice(tb * 512, (tb + 1) * 512)
                    # q then k: 2 waves of 4 pair-groups, chunk-outer accum
                    for qk, (w_t, xoff, boff, dst) in enumerate(
                            ((wq, XQ_OFF, 0, qT2), (wk, XK_OFF, 8, kT2))):
                        xcs = []
                        for c in range(DC):
                            x_c = xpool.tile([P, 512], fp16, tag=f"x{qk}",
                                             name=f"x{qk}_{tb}_{c}")
                            nc.sync.dma_start(
                                x_c[:], xblob[xoff + c * P:xoff + (c + 1) * P,
                                              ts_])
                            xcs.append(x_c)
                        for wave in range(2):
                            ps4 = alloc4(f"psqk{tb}_{qk}_{wave}")
                            for c in range(DC):
                                for j in range(4):
                                    p = wave * 4 + j
                                    nc.tensor.matmul(
                                        ps4[j][:],
                                        w_t[:, c, p * P:(p + 1) * P],
                                        xcs[c][:], start=(c == 0),
                                        stop=(c == DC - 1))
                            for j in range(4):
                                p = wave * 4 + j
                                nc.vector.tensor_scalar_add(
                                    dst[p][:, ts_], ps4[j][:],
                                    bqk[:, boff + p:boff + p + 1])
                    # v: 2 halves x 4 t-tile groups, chunk-outer
                    xvs = []
                    for c in range(DC):
                        xv_c = xpool.tile([P, 512], fp16, tag="xv",
                                          name=f"xv_{tb}_{c}")
                        nc.sync.dma_start(
                            xv_c[:], xblob[XV_OFF + c * P:XV_OFF + (c + 1) * P,
                                           ts_])
                        xvs.append(xv_c)
                    for half in range(2):
                        ps4 = alloc4(f"psv{tb}_{half}")
                        for c in range(DC):
                            for ti in range(4):
                                nc.tensor.matmul(
                                    ps4[ti][:], xvs[c][:, ti * P:(ti + 1) * P],
                                    wv[:, c, half * 512:(half + 1) * 512],
                                    start=(c == 0), stop=(c == DC - 1))
                        for ti in range(4):
                            tt = tb * 4 + ti
                            nc.vector.tensor_copy(
                                V_aug[:, tt, half * 8:(half + 1) * 8, 0:64],
                                ps4[ti][:].rearrange("p (h d) -> p h d", d=64))

                    # early attention for (t-block 0, pair 0): its PV psum is
                    # idle during phase A and every dep of s-chunk quarter tb
                    # is produced by A(tb) — run it here so ACT starts early
                    # instead of idling through all projections.
                    if tb == 0:
                        pv_e0 = psPV.tile([65, 512], f32, tag="pv0",
                                          name="pv0_0_0")
                        pv_e1 = psPV.tile([65, 512], f32, tag="pv1",
                                          name="pv1_0_0")
                    for sc in range(4 * tb, 4 * tb + 4):
                        ss = slice(sc * P, (sc + 1) * P)
                        ps_s = psS.tile([P, 1024], f32, tag="scores",
                                        name=f"ps_s_0_0_{sc}")
                        nc.tensor.matmul(
                            ps_s[:, 0:512], qT2[0][0:64, ss],
                            kT2[0][0:64, 0:512], start=True, stop=True,
                            tile_position=(0, 0))
                        nc.tensor.matmul(
                            ps_s[:, 512:1024], qT2[0][64:128, ss],
                            kT2[0][64:128, 0:512], start=True, stop=True,
                            tile_position=(64, 0))
                        p_t = pearly.tile([P, 1024], fp16, tag="pe",
                                          name=f"pe_{sc}")
                        nc.scalar.activation(p_t[:], ps_s[:], AF.Exp,
                                             scale=0.125)
                        nc.tensor.matmul(
                            pv_e0[:], V_aug[:, sc, 0, :], p_t[:, 0:512],
                            start=(sc == 0), stop=(sc == SC - 1))
                        nc.tensor.matmul(
                            pv_e1[:], V_aug[:, sc, 1, :], p_t[:, 512:1024],
                            start=(sc == 0), stop=(sc == SC - 1))

            # ---- phases B+C per t-block ----
            with tc.tile_pool(name="ppool", bufs=6) as ppool, \
                 tc.tile_pool(name="rbpool", bufs=2) as rbpool, \
                 tc.tile_pool(name="ctxpool", bufs=2) as ctxpool, \
                 tc.tile_pool(name="wopool", bufs=1) as wopool, \
                 tc.tile_pool(name="opool", bufs=3) as opool:
                    wo = wopool.tile([P, NP, D], fp16, name="wo")
                    for p in range(NP):
                        nc.sync.dma_start(
                            wo[:, p], wblob[WVO_OFF + p * P:WVO_OFF + (p + 1) * P,
                                            1024:2048])

                    def flush_evac(pend):
                        # normalize pair into its ctx tile:
                        # ctx[h] = pv[h][0:64] * bcast(1 / pv[h][64])
                        tb, p, pv0, pv1, ctx_p = pend
                        for h, pv in ((0, pv0), (1, pv1)):
                            r_t = rbpool.tile([1, 512], f32r, tag="r",
                                              name=f"r_{tb}_{p}_{h}")
                            with nc.allow_low_precision(reason="softmax recip"):
                                nc.vector.reciprocal(r_t[:], pv[64:65, :])
                            ps_rb = psProj.tile([64, 512], f32, tag="proj",
                                                name=f"ps_rb_{tb}_{p}_{h}")
                            nc.tensor.matmul(ps_rb[:], ones64[:], r_t[:],
                                             start=True, stop=True)
                            rb_s = rbpool.tile([64, 512], f32, tag="rb",
                                               name=f"rb_{tb}_{p}_{h}")
                            nc.vector.tensor_copy(rb_s[:], ps_rb[:])
                            nc.vector.tensor_tensor(
                                ctx_p[h * 64:(h + 1) * 64, :],
                                pv[0:64, :], rb_s[:], MULT)

                    def emit_wo_chunk(wtb, wctx, ti, ob):
                        # one [128t, 512o] WO output tile of t-block wtb
                        ps_o = psProj.tile([P, 512], f32, tag="proj",
                                           name=f"ps_o_{wtb}_{ti}_{ob}")
                        for p in range(NP):
                            nc.tensor.matmul(
                                ps_o[:], wctx[p][:, ti * P:(ti + 1) * P],
                                wo[:, p, ob * 512:(ob + 1) * 512],
                                start=(p == 0), stop=(p == NP - 1))
                        o_t = opool.tile([P, 512], fp16, tag="o",
                                         name=f"o_{wtb}_{ti}_{ob}")
                        nc.vector.tensor_copy(o_t[:], ps_o[:])
                        nc.sync.dma_start(
                            out[wtb * 512 + ti * P: wtb * 512 + (ti + 1) * P,
                                ob * 512:(ob + 1) * 512], o_t[:])

                    # (t-block 0, pair 0) already ran during phase A; seed its
                    # deferred evacuation so pair 1's sc==2 flush handles it.
                    pending = (0, 0, pv_e0, pv_e1, ctx_00)
                    pending_wo = None     # (tb, ctx_tb) whose WO is deferred
                    for tb in range(NTB):
                        ts_ = slice(tb * 512, (tb + 1) * 512)
                        ctx_tb = [ctx_00] if tb == 0 else []
                        for p in range(NP):
                            if tb == 0 and p == 0:
                                continue
                            # -- B: attention for (pair p, t-block tb) --
                            pv0 = psPV.tile([65, 512], f32, tag="pv0",
                                            name=f"pv0_{tb}_{p}")
                            pv1 = psPV.tile([65, 512], f32, tag="pv1",
                                            name=f"pv1_{tb}_{p}")
                            # Defer the previous pair's PV-psum evacuation (and
                            # the previous t-block's WO chunks) past this
                            # pair's first score/exp groups so ACT stays fed
                            # while PE runs the evac/WO work in its slack.
                            stash = []
                            for sc in range(SC):
                                ss = slice(sc * P, (sc + 1) * P)
                                ps_s = psS.tile([P, 1024], f32, tag="scores",
                                                name=f"ps_s_{tb}_{p}_{sc}")
                                nc.tensor.matmul(
                                    ps_s[:, 0:512], qT2[p][0:64, ss],
                                    kT2[p][0:64, ts_], start=True, stop=True,
                                    tile_position=(0, 0))
                                nc.tensor.matmul(
                                    ps_s[:, 512:1024], qT2[p][64:128, ss],
                                    kT2[p][64:128, ts_], start=True, stop=True,
                                    tile_position=(64, 0))
                                p_t = ppool.tile([P, 1024], fp16, tag="p",
                                                 name=f"p_{tb}_{p}_{sc}")
                                nc.scalar.activation(p_t[:], ps_s[:], AF.Exp,
                                                     scale=0.125)
                                if pending is not None and sc < 2:
                                    stash.append((sc, p_t))
                                    continue
                                if pending is not None and sc == 2:
                                    flush_evac(pending)
                                    pending = None
                                for s0, pt0 in stash:
                                    nc.tensor.matmul(
                                        pv0[:], V_aug[:, s0, 2 * p, :],
                                        pt0[:, 0:512],
                                        start=(s0 == 0), stop=False)
                                    nc.tensor.matmul(
                                        pv1[:], V_aug[:, s0, 2 * p + 1, :],
                                        pt0[:, 512:1024],
                                        start=(s0 == 0), stop=False)
                                stash = []
                                nc.tensor.matmul(
                                    pv0[:], V_aug[:, sc, 2 * p, :],
                                    p_t[:, 0:512],
                                    start=(sc == 0), stop=(sc == SC - 1))
                                nc.tensor.matmul(
                                    pv1[:], V_aug[:, sc, 2 * p + 1, :],
                                    p_t[:, 512:1024],
                                    start=(sc == 0), stop=(sc == SC - 1))
                                # sprinkle the previous t-block's 8 WO
                                # chunks across pairs 0-1, every other sc,
                                # to stay under the ACT rate per slot
                                if (pending_wo is not None and p <= 1
                                        and 2 <= sc <= 9 and (sc % 2) == 0):
                                    widx = p * 4 + (sc - 2) // 2
                                    emit_wo_chunk(pending_wo[0], pending_wo[1],
                                                  widx // 2, widx % 2)
                                    if widx == 7:
                                        pending_wo = None
                            ctx_p = ctxpool.tile([P, 512], fp16, tag=f"ctx{p}",
                                                 name=f"ctx_{tb}_{p}")
                            pending = (tb, p, pv0, pv1, ctx_p)
                            ctx_tb.append(ctx_p)
                        pending_wo = (tb, ctx_tb)

                    # tail: last pair's evac + last t-block's WO
                    if pending is not None:
                        flush_evac(pending)
                        pending = None
                    if pending_wo is not None:
                        for ti in range(4):
                            for ob in range(2):
                                emit_wo_chunk(pending_wo[0], pending_wo[1],
                                              ti, ob)
                        pending_wo = None
    _nc_cache[0] = nc
    return nc


# ---------------------------------------------------------------- host side
def make_in_maps(keys, queries, values, WK_w, WK_b, WQ_w, WQ_b, WV_w, WV_b,
                 WO_w):
    keys = np.asarray(keys, dtype=np.float32)
    queries = np.asarray(queries, dtype=np.float32)
    values = np.asarray(values, dtype=np.float32)

    wblob = np.empty((2176, 2048), np.float16)
    wblob[WQK_OFF:WQK_OFF + 1024, 0:1024] = np.asarray(WQ_w, np.float32).T
    wblob[WQK_OFF:WQK_OFF + 1024, 1024:2048] = np.asarray(WK_w, np.float32).T
    wblob[WVO_OFF:WVO_OFF + 1024, 0:1024] = np.asarray(WV_w, np.float32).T
    wblob[WVO_OFF:WVO_OFF + 1024, 1024:2048] = np.asarray(WO_w, np.float32).T
    wblob[BIAS_OFF:BIAS_OFF + P, :] = 0
    wblob[BIAS_OFF:BIAS_OFF + P, 0:8] = \
        np.asarray(WQ_b, np.float32).reshape(NP, P).T
    wblob[BIAS_OFF:BIAS_OFF + P, 8:16] = \
        np.asarray(WK_b, np.float32).reshape(NP, P).T

    in_maps = []
    for b in range(N_CORES):
        xblob = np.empty((3072, 2048), np.float16)
        xblob[XQ_OFF:XQ_OFF + 1024] = queries[b].T
        xblob[XK_OFF:XK_OFF + 1024] = keys[b].T
        xblob[XV_OFF:XV_OFF + 1024] = values[b].T
        in_maps.append({"xblob": xblob, "wblob": wblob})
    return in_maps


def kernel(keys, queries, values, pad_mask, WK_w, WK_b, WQ_w, WQ_b, WV_w, WV_b,
           WO_w, WO_b):
    nc = build_nc()
    in_maps = make_in_maps(keys, queries, values, WK_w, WK_b, WQ_w, WQ_b,
                           WV_w, WV_b, WO_w)
    res = run_bass_kernel_spmd(nc, in_maps, list(range(N_CORES)))
    # free-dim biases folded on host: WO_b directly; WV_b exactly via
    # WV_b @ WO_w^T  (attention rows sum to 1).
    bias = (np.asarray(WO_b, np.float64)
            + np.asarray(WV_b, np.float64) @ np.asarray(WO_w, np.float64).T)
    out = np.empty((B, T, D), np.float32)
    for b in range(B):
        out[b] = (res.results[b]["out"].astype(np.float64) + bias
                  ).astype(np.float32)
    return out


# revision 14
# speedup vs baseline: 27.0464x; 3.2955x over previous
"""Bass/Trainium2 kernel for nn_CasualSelfAttention (B=4, T=2048, D=1024, H=16, dk=64).

Sharding: pure batch-parallel over 4 cores; each core computes all 16 heads of
one batch element end-to-end (QKV projections, attention, WO). No partial sums
across cores -> each core's output is exact (host folds the free-dim biases).

Inputs are packed into two fp16 blobs whose content is IDENTICAL on every core
(so a host harness can pass them replicated / cached across calls); each core
selects its batch via a partition-id-dependent dynamic DMA offset:
  xblob [12288, 2048]: batch b at rows b*3072: 0:1024 xq^T, 1024:2048 xk^T,
        2048:3072 xv^T
  wblob [2176, 2048]: rows 0:1024 = [wq^T | wk^T], rows 1024:2048 =
        [wv^T | wo^T], rows 2048:2176 cols 0:16 = [bq | bk]

All big matmuls run in fp16 operands (fp32 PSUM accumulation -> only operand
quantization error, ~2^-11). Softmax sums use an augmented-V ones column
(M=65 PV matmuls); normalization uses a DVE reciprocal plus a K=1 PE
broadcast matmul.
"""
import sys
import os

sys.path.insert(0, '/opt/trn_rl_repo')

import numpy as np
import orjson

import concourse.bass as bass
import concourse.tile as tile
import concourse.mybir as mybir
from concourse.bass_utils import run_bass_kernel_spmd
from concourse.ap import AP

# ---------------------------------------------------------------- waitsplit
# The walrus build in this container accepts at most ONE semaphore wait per
# engine instruction.  Tile emits multi-wait sync_info; split the extras into
# single-wait NoOps on the same engine stream (in-order => semantically equal).
_ws_counter = [0]


_SELF_WAIT_ENGINES = ("Activation", "DVE")


def _split_instruction_waits(inst, out_list):
    si = inst.get("sync_info")
    if not si or not si.get("on_wait"):
        out_list.append(inst)
        return
    waits = si["on_wait"]
    # ACT/DVE execute strictly in order, so a compute instruction's wait on
    # its OWN engine's semaphore (slot-reuse WAW vs an older instruction on
    # the same engine) is always already satisfied — drop it instead of
    # spending a NoOp dispatch on the bottleneck ACT stream.
    eng = inst.get("engine")
    if (eng in _SELF_WAIT_ENGINES
            and inst.get("opcode") not in ("Drain", "EventSemaphore", "NoOp")):
        kept = [w for w in waits
                if w.get("ant_name", "").rsplit("_", 1)[0] != eng]
        if kept != waits:
            si = dict(si)
            si["on_wait"] = kept
            inst = dict(inst)
            inst["sync_info"] = si
            waits = kept
    if len(waits) <= 1:
        out_list.append(inst)
        return
    for w in waits[:-1]:
        _ws_counter[0] += 1
        out_list.append({
            "debug": inst.get("debug", 0),
            "engine": inst.get("engine"),
            "ins": [],
            "name": f"I-wsplit-{_ws_counter[0]}",
            "opcode": "NoOp",
            "outs": [],
            "sync_info": {"on_update": [], "on_wait": [w]},
        })
    si = dict(si)
    si["on_wait"] = [waits[-1]]
    inst = dict(inst)
    inst["sync_info"] = si
    out_list.append(inst)


def fix_multiwait_json(bir_bytes):
    d = orjson.loads(bir_bytes)
    for fn in d["functions"]:
        for bb in fn["blocks"]:
            new = []
            for inst in bb["instructions"]:
                _split_instruction_waits(inst, new)
            bb["instructions"] = new
    return orjson.dumps(d)


class WaitSplitBass(bass.Bass):
    def to_json_bytes(self):
        return fix_multiwait_json(super().to_json_bytes())


# ---------------------------------------------------------------- kernel build
P = 128
B, T, D = 4, 2048, 1024
N_CORES = 4           # batch-parallel
NH_LOC = 16           # heads per core (all of them)
NP = NH_LOC // 2      # head pairs per core = 8
DK = 64
DC = D // P           # 8 d_model chunks
SC = T // P           # 16 s-chunks
NTB = T // 512        # 4 t-blocks
f32 = mybir.dt.float32
f32r = mybir.dt.float32r
fp16 = mybir.dt.float16
AF = mybir.ActivationFunctionType
MULT = mybir.AluOpType.mult

# xblob row offsets (within a core's 3072-row batch block)
XQ_OFF, XK_OFF, XV_OFF = 0, 1024, 2048
XBLOB_CORE_ROWS = 3072
XBLOB_CORE_ELEMS = XBLOB_CORE_ROWS * 2048
# wblob row offsets
WQK_OFF = 0       # cols 0:1024 wq^T, 1024:2048 wk^T
WVO_OFF = 1024    # cols 0:1024 wv^T, 1024:2048 wo^T
BIAS_OFF = 2048   # rows 2048:2176, cols 0:8 bq, 8:16 bk

_nc_cache = [None]


def build_nc():
    if _nc_cache[0] is not None:
        return _nc_cache[0]
    nc = WaitSplitBass()
    xblob = nc.dram_tensor("xblob", [N_CORES * XBLOB_CORE_ROWS, 2048], fp16,
                           kind="ExternalInput")
    wblob = nc.dram_tensor("wblob", [2176, 2048], fp16, kind="ExternalInput")
    out = nc.dram_tensor("out", [T, D], fp16, kind="ExternalOutput")

    with tile.TileContext(nc) as tc:
        # every core receives the identical xblob (all batches); this core's
        # batch block starts at partition_id * XBLOB_CORE_ELEMS
        pid_off = nc.partition_id() * XBLOB_CORE_ELEMS

        def xsrc(xoff, tb):
            # [P, DC, 512] view of this core's x tensor block, t-block tb
            s = xblob[xoff:xoff + 1024, tb * 512:(tb + 1) * 512] \
                .rearrange("(c p) t -> p c t", p=P)
            return AP(tensor=s.tensor, offset=s.offset + pid_off, ap=s.ap)

        with tc.tile_pool(name="persist", bufs=1) as persist, \
             tc.tile_pool(name="psProj", bufs=2, space="PSUM") as psProj, \
             tc.tile_pool(name="psS", bufs=2, space="PSUM") as psS, \
             tc.tile_pool(name="psPV", bufs=1, space="PSUM") as psPV:

            # ---- persistent tiles ----
            qT2 = [persist.tile([P, T], fp16, tag=f"qT2_{p}", name=f"qT2_{p}")
                   for p in range(NP)]
            kT2 = [persist.tile([P, T], fp16, tag=f"kT2_{p}", name=f"kT2_{p}")
                   for p in range(NP)]
            V_aug = persist.tile([P, SC, NH_LOC, 65], fp16, name="V_aug")
            nc.vector.memset(V_aug[:, :, :, 64], 1.0)
            b16 = persist.tile([P, 16], fp16, name="b16")
            nc.sync.dma_start(b16[:], wblob[BIAS_OFF:BIAS_OFF + P, 0:16])
            bqk = persist.tile([P, 16], f32, name="bqk")
            nc.vector.tensor_copy(bqk[:], b16[:])
            ones64 = persist.tile([1, 64], f32r, name="ones64")
            nc.vector.memset(ones64[:].bitcast(f32), 1.0)

            # ---- phase A: projections (weights + X^T streamed per chunk) ----
            # 4 concurrent psum groups: 2 slots borrowed from the (idle)
            # scores pool + 2 from psProj.  8 head-pairs (or 2 v-halves) are
            # processed in 2 waves of 4 groups.
            def alloc4(stem):
                ps = [psS.tile([P, 1024], f32, tag="scores",
                               name=f"{stem}_s{j}")[:, 0:512] for j in range(2)]
                ps += [psProj.tile([P, 512], f32, tag="proj",
                                   name=f"{stem}_p{j}") for j in range(2)]
                return ps

            ctx_00 = persist.tile([P, 512], fp16, name="ctx_00")
            with tc.tile_pool(name="pearly", bufs=2) as pearly, \
                 tc.tile_pool(name="wpool", bufs=1) as wpool, \
                 tc.tile_pool(name="xpool", bufs=2) as xpool:
                wq = wpool.tile([P, DC, 1024], fp16, tag="wq", name="wq")
                wk = wpool.tile([P, DC, 1024], fp16, tag="wk", name="wk")
                wv = wpool.tile([P, DC, 1024], fp16, tag="wv", name="wv")
                for c in range(DC):
                    nc.sync.dma_start(
                        wq[:, c], wblob[WQK_OFF + c * P:WQK_OFF + (c + 1) * P,
                                        0:1024])
                    nc.sync.dma_start(
                        wk[:, c], wblob[WQK_OFF + c * P:WQK_OFF + (c + 1) * P,
                                        1024:2048])
                    nc.sync.dma_start(
                        wv[:, c], wblob[WVO_OFF + c * P:WVO_OFF + (c + 1) * P,
                                        0:1024])

                for tb in range(NTB):
                    ts_ = slice(tb * 512, (tb + 1) * 512)
                    # q then k: 2 waves of 4 pair-groups, chunk-outer accum
                    for qk, (w_t, xoff, boff, dst) in enumerate(
                            ((wq, XQ_OFF, 0, qT2), (wk, XK_OFF, 8, kT2))):
                        x_all = xpool.tile([P, DC, 512], fp16, tag=f"x{qk}",
                                           name=f"x{qk}_{tb}")
                        nc.sync.dma_start(x_all[:], xsrc(xoff, tb))
                        xcs = [x_all[:, c, :] for c in range(DC)]
                        for wave in range(2):
                            ps4 = alloc4(f"psqk{tb}_{qk}_{wave}")
                            for c in range(DC):
                                for j in range(4):
                                    p = wave * 4 + j
                                    nc.tensor.matmul(
                                        ps4[j][:],
                                        w_t[:, c, p * P:(p + 1) * P],
                                        xcs[c][:], start=(c == 0),
                                        stop=(c == DC - 1))
                            for j in range(4):
                                p = wave * 4 + j
                                nc.vector.tensor_scalar_add(
                                    dst[p][:, ts_], ps4[j][:],
                                    bqk[:, boff + p:boff + p + 1])
                    # v: 2 halves x 4 t-tile groups, chunk-outer
                    xv_all = xpool.tile([P, DC, 512], fp16, tag="xv",
                                        name=f"xv_{tb}")
                    nc.sync.dma_start(xv_all[:], xsrc(XV_OFF, tb))
                    xvs = [xv_all[:, c, :] for c in range(DC)]
                    for half in range(2):
                        ps4 = alloc4(f"psv{tb}_{half}")
                        for c in range(DC):
                            for ti in range(4):
                                nc.tensor.matmul(
                                    ps4[ti][:], xvs[c][:, ti * P:(ti + 1) * P],
                                    wv[:, c, half * 512:(half + 1) * 512],
                                    start=(c == 0), stop=(c == DC - 1))
                        for ti in range(4):
                            tt = tb * 4 + ti
                            nc.vector.tensor_copy(
                                V_aug[:, tt, half * 8:(half + 1) * 8, 0:64],
                                ps4[ti][:].rearrange("p (h d) -> p h d", d=64))

                    # early attention for (t-block 0, pair 0): its PV psum is
                    # idle during phase A and every dep of s-chunk quarter tb
                    # is produced by A(tb) — run it here so ACT starts early
                    # instead of idling through all projections.
                    if tb == 0:
                        pv_e0 = psPV.tile([65, 512], f32, tag="pv0",
                                          name="pv0_0_0")
                        pv_e1 = psPV.tile([65, 512], f32, tag="pv1",
                                          name="pv1_0_0")
                    for sc in range(4 * tb, 4 * tb + 4):
                        ss = slice(sc * P, (sc + 1) * P)
                        ps_s = psS.tile([P, 1024], f32, tag="scores",
                                        name=f"ps_s_0_0_{sc}")
                        nc.tensor.matmul(
                            ps_s[:, 0:512], qT2[0][0:64, ss],
                            kT2[0][0:64, 0:512], start=True, stop=True,
                            tile_position=(0, 0))
                        nc.tensor.matmul(
                            ps_s[:, 512:1024], qT2[0][64:128, ss],
                            kT2[0][64:128, 0:512], start=True, stop=True,
                            tile_position=(64, 0))
                        p_t = pearly.tile([P, 1024], fp16, tag="pe",
                                          name=f"pe_{sc}")
                        nc.scalar.activation(p_t[:], ps_s[:], AF.Exp,
                                             scale=0.125)
                        nc.tensor.matmul(
                            pv_e0[:], V_aug[:, sc, 0, :], p_t[:, 0:512],
                            start=(sc == 0), stop=(sc == SC - 1))
                        nc.tensor.matmul(
                            pv_e1[:], V_aug[:, sc, 1, :], p_t[:, 512:1024],
                            start=(sc == 0), stop=(sc == SC - 1))

            # ---- phases B+C per t-block ----
            with tc.tile_pool(name="ppool", bufs=6) as ppool, \
                 tc.tile_pool(name="rbpool", bufs=2) as rbpool, \
                 tc.tile_pool(name="ctxpool", bufs=2) as ctxpool, \
                 tc.tile_pool(name="wopool", bufs=1) as wopool, \
                 tc.tile_pool(name="opool", bufs=3) as opool:
                    wo = wopool.tile([P, NP, D], fp16, name="wo")
                    for p in range(NP):
                        nc.sync.dma_start(
                            wo[:, p], wblob[WVO_OFF + p * P:WVO_OFF + (p + 1) * P,
                                            1024:2048])

                    def flush_evac(pend):
                        # normalize pair into its ctx tile:
                        # ctx[h] = pv[h][0:64] * bcast(1 / pv[h][64])
                        tb, p, pv0, pv1, ctx_p = pend
                        for h, pv in ((0, pv0), (1, pv1)):
                            r_t = rbpool.tile([1, 512], f32r, tag="r",
                                              name=f"r_{tb}_{p}_{h}")
                            with nc.allow_low_precision(reason="softmax recip"):
                                nc.vector.reciprocal(r_t[:], pv[64:65, :])
                            ps_rb = psProj.tile([64, 512], f32, tag="proj",
                                                name=f"ps_rb_{tb}_{p}_{h}")
                            nc.tensor.matmul(ps_rb[:], ones64[:], r_t[:],
                                             start=True, stop=True)
                            rb_s = rbpool.tile([64, 512], f32, tag="rb",
                                               name=f"rb_{tb}_{p}_{h}")
                            nc.vector.tensor_copy(rb_s[:], ps_rb[:])
                            nc.vector.tensor_tensor(
                                ctx_p[h * 64:(h + 1) * 64, :],
                                pv[0:64, :], rb_s[:], MULT)

                    def emit_wo_chunk(wtb, wctx, ti, ob):
                        # one [128t, 512o] WO output tile of t-block wtb
                        ps_o = psProj.tile([P, 512], f32, tag="proj",
                                           name=f"ps_o_{wtb}_{ti}_{ob}")
                        for p in range(NP):
                            nc.tensor.matmul(
                                ps_o[:], wctx[p][:, ti * P:(ti + 1) * P],
                                wo[:, p, ob * 512:(ob + 1) * 512],
                                start=(p == 0), stop=(p == NP - 1))
                        o_t = opool.tile([P, 512], fp16, tag="o",
                                         name=f"o_{wtb}_{ti}_{ob}")
                        nc.vector.tensor_copy(o_t[:], ps_o[:])
                        nc.sync.dma_start(
                            out[wtb * 512 + ti * P: wtb * 512 + (ti + 1) * P,
                                ob * 512:(ob + 1) * 512], o_t[:])

                    # (t-block 0, pair 0) already ran during phase A; seed its
                    # deferred evacuation so pair 1's sc==2 flush handles it.
                    pending = (0, 0, pv_e0, pv_e1, ctx_00)
                    pending_wo = None     # (tb, ctx_tb) whose WO is deferred
                    for tb in range(NTB):
                        ts_ = slice(tb * 512, (tb + 1) * 512)
                        ctx_tb = [ctx_00] if tb == 0 else []
                        for p in range(NP):
                            if tb == 0 and p == 0:
                                continue
                            # -- B: attention for (pair p, t-block tb) --
                            pv0 = psPV.tile([65, 512], f32, tag="pv0",
                                            name=f"pv0_{tb}_{p}")
                            pv1 = psPV.tile([65, 512], f32, tag="pv1",
                                            name=f"pv1_{tb}_{p}")
                            # Defer the previous pair's PV-psum evacuation (and
                            # the previous t-block's WO chunks) past this
                            # pair's first score/exp groups so ACT stays fed
                            # while PE runs the evac/WO work in its slack.
                            stash = []
                            for sc in range(SC):
                                ss = slice(sc * P, (sc + 1) * P)
                                ps_s = psS.tile([P, 1024], f32, tag="scores",
                                                name=f"ps_s_{tb}_{p}_{sc}")
                                nc.tensor.matmul(
                                    ps_s[:, 0:512], qT2[p][0:64, ss],
                                    kT2[p][0:64, ts_], start=True, stop=True,
                                    tile_position=(0, 0))
                                nc.tensor.matmul(
                                    ps_s[:, 512:1024], qT2[p][64:128, ss],
                                    kT2[p][64:128, ts_], start=True, stop=True,
                                    tile_position=(64, 0))
                                p_t = ppool.tile([P, 1024], fp16, tag="p",
                                                 name=f"p_{tb}_{p}_{sc}")
                                nc.scalar.activation(p_t[:], ps_s[:], AF.Exp,
                                                     scale=0.125)
                                if pending is not None and sc < 2:
                                    stash.append((sc, p_t))
                                    continue
                                if pending is not None and sc == 2:
                                    flush_evac(pending)
                                    pending = None
                                for s0, pt0 in stash:
                                    nc.tensor.matmul(
                                        pv0[:], V_aug[:, s0, 2 * p, :],
                                        pt0[:, 0:512],
                                        start=(s0 == 0), stop=False)
                                    nc.tensor.matmul(
                                        pv1[:], V_aug[:, s0, 2 * p + 1, :],
                                        pt0[:, 512:1024],
                                        start=(s0 == 0), stop=False)
                                stash = []
                                nc.tensor.matmul(
                                    pv0[:], V_aug[:, sc, 2 * p, :],
                                    p_t[:, 0:512],
                                    start=(sc == 0), stop=(sc == SC - 1))
                                nc.tensor.matmul(
                                    pv1[:], V_aug[:, sc, 2 * p + 1, :],
                                    p_t[:, 512:1024],
                                    start=(sc == 0), stop=(sc == SC - 1))
                                # sprinkle the previous t-block's 8 WO
                                # chunks across pairs 0-1, every other sc,
                                # to stay under the ACT rate per slot
                                if (pending_wo is not None and p <= 1
                                        and 2 <= sc <= 9 and (sc % 2) == 0):
                                    widx = p * 4 + (sc - 2) // 2
                                    emit_wo_chunk(pending_wo[0], pending_wo[1],
                                                  widx // 2, widx % 2)
                                    if widx == 7:
                                        pending_wo = None
                            ctx_p = ctxpool.tile([P, 512], fp16, tag=f"ctx{p}",
                                                 name=f"ctx_{tb}_{p}")
                            pending = (tb, p, pv0, pv1, ctx_p)
                            ctx_tb.append(ctx_p)
                        pending_wo = (tb, ctx_tb)

                    # tail: last pair's evac + last t-block's WO
                    if pending is not None:
                        flush_evac(pending)
                        pending = None
                    if pending_wo is not None:
                        for ti in range(4):
                            for ob in range(2):
                                emit_wo_chunk(pending_wo[0], pending_wo[1],
                                              ti, ob)
                        pending_wo = None
    _nc_cache[0] = nc
    return nc


# ---------------------------------------------------------------- host side
def make_in_maps(keys, queries, values, WK_w, WK_b, WQ_w, WQ_b, WV_w, WV_b,
                 WO_w):
    keys = np.asarray(keys, dtype=np.float32)
    queries = np.asarray(queries, dtype=np.float32)
    values = np.asarray(values, dtype=np.float32)

    wblob = np.empty((2176, 2048), np.float16)
    wblob[WQK_OFF:WQK_OFF + 1024, 0:1024] = np.asarray(WQ_w, np.float32).T
    wblob[WQK_OFF:WQK_OFF + 1024, 1024:2048] = np.asarray(WK_w, np.float32).T
    wblob[WVO_OFF:WVO_OFF + 1024, 0:1024] = np.asarray(WV_w, np.float32).T
    wblob[WVO_OFF:WVO_OFF + 1024, 1024:2048] = np.asarray(WO_w, np.float32).T
    wblob[BIAS_OFF:BIAS_OFF + P, :] = 0
    wblob[BIAS_OFF:BIAS_OFF + P, 0:8] = \
        np.asarray(WQ_b, np.float32).reshape(NP, P).T
    wblob[BIAS_OFF:BIAS_OFF + P, 8:16] = \
        np.asarray(WK_b, np.float32).reshape(NP, P).T

    xblob = np.empty((N_CORES * XBLOB_CORE_ROWS, 2048), np.float16)
    for b in range(N_CORES):
        r = b * XBLOB_CORE_ROWS
        xblob[r + XQ_OFF:r + XQ_OFF + 1024] = queries[b].T
        xblob[r + XK_OFF:r + XK_OFF + 1024] = keys[b].T
        xblob[r + XV_OFF:r + XV_OFF + 1024] = values[b].T
    return [{"xblob": xblob, "wblob": wblob} for _ in range(N_CORES)]


def kernel(keys, queries, values, pad_mask, WK_w, WK_b, WQ_w, WQ_b, WV_w, WV_b,
           WO_w, WO_b):
    nc = build_nc()
    in_maps = make_in_maps(keys, queries, values, WK_w, WK_b, WQ_w, WQ_b,
                           WV_w, WV_b, WO_w)
    res = run_bass_kernel_spmd(nc, in_maps, list(range(N_CORES)))
    # free-dim biases folded on host: WO_b directly; WV_b exactly via
    # WV_b @ WO_w^T  (attention rows sum to 1).
    bias = (np.asarray(WO_b, np.float64)
            + np.asarray(WV_b, np.float64) @ np.asarray(WO_w, np.float64).T)
    out = np.empty((B, T, D), np.float32)
    for b in range(B):
        out[b] = (res.results[b]["out"].astype(np.float64) + bias
                  ).astype(np.float32)
    return out


# revision 20
# speedup vs baseline: 51.2288x; 1.8941x over previous
"""Bass/Trainium2 kernel for nn_CasualSelfAttention (B=4, T=2048, D=1024, H=16, dk=64).

Sharding: batch (4) x head-group (2) = 8 cores. Each core computes 8 heads of one
batch element end-to-end (QKV projections, attention, WO partial product); the
host sums the two head-group partials per batch and folds the free-dim biases.

Inputs are packed into two fp16 blobs whose content is IDENTICAL on every core
(so a host harness can pass them replicated / cached across calls); each core
selects its batch via a partition-id-dependent dynamic DMA offset
(b = pid//2 rows into xblob) and its head-group via a column/row shift
(g = pid%2) on the weight slices:
  xblob [12288, 2048]: batch b at rows b*3072: 0:1024 xq^T, 1024:2048 xk^T,
        2048:3072 xv^T
  wblob [2176, 2048]: rows 0:1024 = [wq^T | wk^T], rows 1024:2048 =
        [wv^T | wo^T], rows 2048:2176 cols 0:16 = [bq | bk]  (8 pairs each)

All big matmuls run on fp16 operands (fp32 PSUM accumulation -> only operand
quantization error, ~2^-11). Softmax sums use an augmented-V ones column
(M=65 PV matmuls); normalization uses a DVE reciprocal plus a K=1 PE
broadcast matmul.
"""
import sys
import os

sys.path.insert(0, '/opt/trn_rl_repo')

import numpy as np
import orjson

import concourse.bass as bass
import concourse.tile as tile
import concourse.mybir as mybir
from concourse.bass_utils import run_bass_kernel_spmd
from concourse.ap import AP

# ---------------------------------------------------------------- waitsplit
# The walrus build in this container accepts at most ONE semaphore wait per
# engine instruction.  Tile emits multi-wait sync_info; split the extras into
# single-wait NoOps on the same engine stream (in-order => semantically equal).
_ws_counter = [0]


_SELF_WAIT_ENGINES = ("Activation", "DVE")


def _split_instruction_waits(inst, out_list):
    si = inst.get("sync_info")
    if not si or not si.get("on_wait"):
        out_list.append(inst)
        return
    waits = si["on_wait"]
    # ACT/DVE execute strictly in order, so a compute instruction's wait on
    # its OWN engine's semaphore (slot-reuse WAW vs an older instruction on
    # the same engine) is always already satisfied — drop it instead of
    # spending a NoOp dispatch on the bottleneck ACT stream.
    eng = inst.get("engine")
    if (eng in _SELF_WAIT_ENGINES
            and inst.get("opcode") not in ("Drain", "EventSemaphore", "NoOp")):
        kept = [w for w in waits
                if w.get("ant_name", "").rsplit("_", 1)[0] != eng]
        if kept != waits:
            si = dict(si)
            si["on_wait"] = kept
            inst = dict(inst)
            inst["sync_info"] = si
            waits = kept
    if len(waits) <= 1:
        out_list.append(inst)
        return
    for w in waits[:-1]:
        _ws_counter[0] += 1
        out_list.append({
            "debug": inst.get("debug", 0),
            "engine": inst.get("engine"),
            "ins": [],
            "name": f"I-wsplit-{_ws_counter[0]}",
            "opcode": "NoOp",
            "outs": [],
            "sync_info": {"on_update": [], "on_wait": [w]},
        })
    si = dict(si)
    si["on_wait"] = [waits[-1]]
    inst = dict(inst)
    inst["sync_info"] = si
    out_list.append(inst)


def fix_multiwait_json(bir_bytes):
    d = orjson.loads(bir_bytes)
    for fn in d["functions"]:
        for bb in fn["blocks"]:
            new = []
            for inst in bb["instructions"]:
                _split_instruction_waits(inst, new)
            bb["instructions"] = new
    return orjson.dumps(d)


class WaitSplitBass(bass.Bass):
    def to_json_bytes(self):
        return fix_multiwait_json(super().to_json_bytes())


# ---------------------------------------------------------------- kernel build
P = 128
B, T, D = 4, 2048, 1024
N_CORES = 8           # batch (4) x head-group (2)
NH_LOC = 8            # heads per core
NP = NH_LOC // 2      # head pairs per core = 4
DK = 64
DC = D // P           # 8 d_model chunks
SC = T // P           # 16 s-chunks
NTB = T // 512        # 4 t-blocks
f32 = mybir.dt.float32
f32r = mybir.dt.float32r
fp16 = mybir.dt.float16
AF = mybir.ActivationFunctionType
MULT = mybir.AluOpType.mult

# xblob row offsets (within a batch's 3072-row block)
XQ_OFF, XK_OFF, XV_OFF = 0, 1024, 2048
XBLOB_CORE_ROWS = 3072
XBLOB_CORE_ELEMS = XBLOB_CORE_ROWS * 2048
# wblob row offsets
WQK_OFF = 0       # cols 0:1024 wq^T, 1024:2048 wk^T
WVO_OFF = 1024    # cols 0:1024 wv^T, 1024:2048 wo^T
BIAS_OFF = 2048   # rows 2048:2176, cols 0:8 bq (8 pairs), 8:16 bk

_nc_cache = [None]


def build_nc():
    if _nc_cache[0] is not None:
        return _nc_cache[0]
    nc = WaitSplitBass()
    xblob = nc.dram_tensor("xblob", [B * XBLOB_CORE_ROWS, 2048], fp16,
                           kind="ExternalInput")
    wblob = nc.dram_tensor("wblob", [2176, 2048], fp16, kind="ExternalInput")
    out = nc.dram_tensor("out", [T, D], fp16, kind="ExternalOutput")

    with tile.TileContext(nc) as tc:
        # every core receives identical blobs; batch = pid//2, head-group =
        # pid%2 select this core's slices via dynamic AP offsets
        pid = nc.partition_id()
        b_off = (pid // 2) * XBLOB_CORE_ELEMS
        g = pid % 2
        g_col = g * 512           # column shift for wq/wk/wv slices
        g_row = g * (512 * 2048)  # row shift for wo slice
        g_b = g * 4               # bias column shift

        def dyn(s, off):
            return AP(tensor=s.tensor, offset=s.offset + off, ap=s.ap)

        def xsrc(xoff, tb):
            # [P, DC, 512] view of this core's x tensor block, t-block tb
            s = xblob[xoff:xoff + 1024, tb * 512:(tb + 1) * 512] \
                .rearrange("(c p) t -> p c t", p=P)
            return dyn(s, b_off)

        with tc.tile_pool(name="persist", bufs=1) as persist, \
             tc.tile_pool(name="psProj", bufs=2, space="PSUM") as psProj, \
             tc.tile_pool(name="psS", bufs=2, space="PSUM") as psS, \
             tc.tile_pool(name="psPV", bufs=1, space="PSUM") as psPV:

            # ---- persistent tiles ----
            qT2 = [persist.tile([P, T], fp16, tag=f"qT2_{p}", name=f"qT2_{p}")
                   for p in range(NP)]
            kT2 = [persist.tile([P, T], fp16, tag=f"kT2_{p}", name=f"kT2_{p}")
                   for p in range(NP)]
            V_aug = persist.tile([P, SC, NH_LOC, 65], fp16, name="V_aug")
            nc.vector.memset(V_aug[:, :, :, 64], 1.0)
            b16 = persist.tile([P, 8], fp16, name="b16")
            nc.sync.dma_start(b16[:, 0:4],
                              dyn(wblob[BIAS_OFF:BIAS_OFF + P, 0:4], g_b))
            nc.sync.dma_start(b16[:, 4:8],
                              dyn(wblob[BIAS_OFF:BIAS_OFF + P, 8:12], g_b))
            bqk = persist.tile([P, 8], f32, name="bqk")
            nc.vector.tensor_copy(bqk[:], b16[:])
            ones64 = persist.tile([1, 64], f32r, name="ones64")
            nc.vector.memset(ones64[:].bitcast(f32), 1.0)

            # ---- phase A: projections (weights + X^T streamed per chunk) ----
            # 4 concurrent psum groups: 2 slots borrowed from the (idle)
            # scores pool + 2 from psProj.
            def alloc4(stem):
                ps = [psS.tile([P, 1024], f32, tag="scores",
                               name=f"{stem}_s{j}")[:, 0:512] for j in range(2)]
                ps += [psProj.tile([P, 512], f32, tag="proj",
                                   name=f"{stem}_p{j}") for j in range(2)]
                return ps

            ctx_00 = persist.tile([P, 512], fp16, name="ctx_00")
            with tc.tile_pool(name="pearly", bufs=2) as pearly, \
                 tc.tile_pool(name="wpool", bufs=1) as wpool, \
                 tc.tile_pool(name="xpool", bufs=2) as xpool:
                wq = wpool.tile([P, DC, 512], fp16, tag="wq", name="wq")
                wk = wpool.tile([P, DC, 512], fp16, tag="wk", name="wk")
                wv = wpool.tile([P, DC, 512], fp16, tag="wv", name="wv")

                def wsrc(row0, col0):
                    s = wblob[row0:row0 + 1024, col0:col0 + 512] \
                        .rearrange("(c p) t -> p c t", p=P)
                    return dyn(s, g_col)

                nc.sync.dma_start(wq[:], wsrc(WQK_OFF, 0))
                nc.sync.dma_start(wk[:], wsrc(WQK_OFF, 1024))
                nc.sync.dma_start(wv[:], wsrc(WVO_OFF, 0))

                for tb in range(NTB):
                    ts_ = slice(tb * 512, (tb + 1) * 512)
                    # xq+xk+xv rows are adjacent in xblob -> ONE dynamic DMA
                    # per t-block (dynamic DMAs each pin a queue register
                    # pair; the pool is small, so batch aggressively).
                    x3_all = xpool.tile([P, 3 * DC, 512], fp16, tag="x3",
                                        name=f"x3_{tb}")
                    s_x = xblob[0:3 * 1024, tb * 512:(tb + 1) * 512] \
                        .rearrange("(c p) t -> p c t", p=P)
                    nc.sync.dma_start(x3_all[:], dyn(s_x, b_off))
                    # q then k: 4 pair-groups, chunk-outer accumulation
                    for qk, (w_t, boff, dst) in enumerate(
                            ((wq, 0, qT2), (wk, 4, kT2))):
                        x_all = x3_all[:, qk * DC:(qk + 1) * DC, :]
                        ps4 = alloc4(f"psqk{tb}_{qk}")
                        for c in range(DC):
                            for p in range(NP):
                                nc.tensor.matmul(
                                    ps4[p][:], w_t[:, c, p * P:(p + 1) * P],
                                    x_all[:, c, :], start=(c == 0),
                                    stop=(c == DC - 1))
                        for p in range(NP):
                            nc.vector.tensor_scalar_add(
                                dst[p][:, ts_], ps4[p][:],
                                bqk[:, boff + p:boff + p + 1])
                    # v: 4 t-tile groups, chunk-outer
                    xv_all = x3_all[:, 2 * DC:3 * DC, :]
                    ps4 = alloc4(f"psv{tb}")
                    for c in range(DC):
                        for ti in range(4):
                            nc.tensor.matmul(
                                ps4[ti][:], xv_all[:, c, ti * P:(ti + 1) * P],
                                wv[:, c], start=(c == 0), stop=(c == DC - 1))
                    for ti in range(4):
                        tt = tb * 4 + ti
                        nc.vector.tensor_copy(
                            V_aug[:, tt, :, 0:64],
                            ps4[ti][:].rearrange("p (h d) -> p h d", d=64))

                    # early attention for (t-block 0, pair 0): its PV psum is
                    # idle during phase A and every dep of s-chunk quarter tb
                    # is produced by A(tb) — run it here so ACT starts early
                    # instead of idling through all projections.
                    if tb == 0:
                        pv_e0 = psPV.tile([65, 512], f32, tag="pv0",
                                          name="pv0_0_0")
                        pv_e1 = psPV.tile([65, 512], f32, tag="pv1",
                                          name="pv1_0_0")
                    for sc in range(4 * tb, 4 * tb + 4):
                        ss = slice(sc * P, (sc + 1) * P)
                        ps_s = psS.tile([P, 1024], f32, tag="scores",
                                        name=f"ps_s_0_0_{sc}")
                        nc.tensor.matmul(
                            ps_s[:, 0:512], qT2[0][0:64, ss],
                            kT2[0][0:64, 0:512], start=True, stop=True,
                            tile_position=(0, 0))
                        nc.tensor.matmul(
                            ps_s[:, 512:1024], qT2[0][64:128, ss],
                            kT2[0][64:128, 0:512], start=True, stop=True,
                            tile_position=(64, 0))
                        p_t = pearly.tile([P, 1024], fp16, tag="pe",
                                          name=f"pe_{sc}")
                        nc.scalar.activation(p_t[:], ps_s[:], AF.Exp,
                                             scale=0.125)
                        nc.tensor.matmul(
                            pv_e0[:], V_aug[:, sc, 0, :], p_t[:, 0:512],
                            start=(sc == 0), stop=(sc == SC - 1))
                        nc.tensor.matmul(
                            pv_e1[:], V_aug[:, sc, 1, :], p_t[:, 512:1024],
                            start=(sc == 0), stop=(sc == SC - 1))

            # ---- phases B+C per t-block ----
            with tc.tile_pool(name="ppool", bufs=5) as ppool, \
                 tc.tile_pool(name="rbpool", bufs=2) as rbpool, \
                 tc.tile_pool(name="ctxpool", bufs=2) as ctxpool, \
                 tc.tile_pool(name="wopool", bufs=1) as wopool, \
                 tc.tile_pool(name="opool", bufs=3) as opool:
                    wo = wopool.tile([P, NP, D], fp16, name="wo")
                    s_wo = wblob[WVO_OFF:WVO_OFF + NP * P, 1024:2048] \
                        .rearrange("(c p) t -> p c t", p=P)
                    nc.sync.dma_start(wo[:], dyn(s_wo, g_row))

                    def flush_evac(pend):
                        # normalize pair into its ctx tile:
                        # ctx[h] = pv[h][0:64] * bcast(1 / pv[h][64])
                        tb, p, pv0, pv1, ctx_p = pend
                        for h, pv in ((0, pv0), (1, pv1)):
                            r_t = rbpool.tile([1, 512], f32r, tag="r",
                                              name=f"r_{tb}_{p}_{h}")
                            with nc.allow_low_precision(reason="softmax recip"):
                                nc.vector.reciprocal(r_t[:], pv[64:65, :])
                            ps_rb = psProj.tile([64, 512], f32, tag="proj",
                                                name=f"ps_rb_{tb}_{p}_{h}")
                            nc.tensor.matmul(ps_rb[:], ones64[:], r_t[:],
                                             start=True, stop=True)
                            rb_s = rbpool.tile([64, 512], f32, tag="rb",
                                               name=f"rb_{tb}_{p}_{h}")
                            nc.vector.tensor_copy(rb_s[:], ps_rb[:])
                            nc.vector.tensor_tensor(
                                ctx_p[h * 64:(h + 1) * 64, :],
                                pv[0:64, :], rb_s[:], MULT)

                    def emit_wo_chunk(wtb, wctx, ti, ob):
                        # one [128t, 512o] WO output tile of t-block wtb
                        ps_o = psProj.tile([P, 512], f32, tag="proj",
                                           name=f"ps_o_{wtb}_{ti}_{ob}")
                        for p in range(NP):
                            nc.tensor.matmul(
                                ps_o[:], wctx[p][:, ti * P:(ti + 1) * P],
                                wo[:, p, ob * 512:(ob + 1) * 512],
                                start=(p == 0), stop=(p == NP - 1))
                        o_t = opool.tile([P, 512], fp16, tag="o",
                                         name=f"o_{wtb}_{ti}_{ob}")
                        nc.vector.tensor_copy(o_t[:], ps_o[:])
                        nc.sync.dma_start(
                            out[wtb * 512 + ti * P: wtb * 512 + (ti + 1) * P,
                                ob * 512:(ob + 1) * 512], o_t[:])

                    # (t-block 0, pair 0) already ran during phase A; seed its
                    # deferred evacuation so pair 1's sc==2 flush handles it.
                    pending = (0, 0, pv_e0, pv_e1, ctx_00)
                    pending_wo = None     # (tb, ctx_tb) whose WO is deferred
                    for tb in range(NTB):
                        ts_ = slice(tb * 512, (tb + 1) * 512)
                        ctx_tb = [ctx_00] if tb == 0 else []
                        for p in range(NP):
                            if tb == 0 and p == 0:
                                continue
                            # -- B: attention for (pair p, t-block tb) --
                            pv0 = psPV.tile([65, 512], f32, tag="pv0",
                                            name=f"pv0_{tb}_{p}")
                            pv1 = psPV.tile([65, 512], f32, tag="pv1",
                                            name=f"pv1_{tb}_{p}")
                            # Defer the previous pair's PV-psum evacuation (and
                            # the previous t-block's WO chunks) past this
                            # pair's first score/exp groups so ACT stays fed
                            # while PE runs the evac/WO work in its slack.
                            stash = []
                            for sc in range(SC):
                                ss = slice(sc * P, (sc + 1) * P)
                                ps_s = psS.tile([P, 1024], f32, tag="scores",
                                                name=f"ps_s_{tb}_{p}_{sc}")
                                nc.tensor.matmul(
                                    ps_s[:, 0:512], qT2[p][0:64, ss],
                                    kT2[p][0:64, ts_], start=True, stop=True,
                                    tile_position=(0, 0))
                                nc.tensor.matmul(
                                    ps_s[:, 512:1024], qT2[p][64:128, ss],
                                    kT2[p][64:128, ts_], start=True, stop=True,
                                    tile_position=(64, 0))
                                p_t = ppool.tile([P, 1024], fp16, tag="p",
                                                 name=f"p_{tb}_{p}_{sc}")
                                nc.scalar.activation(p_t[:], ps_s[:], AF.Exp,
                                                     scale=0.125)
                                if pending is not None and sc < 2:
                                    stash.append((sc, p_t))
                                    continue
                                if pending is not None and sc == 2:
                                    flush_evac(pending)
                                    pending = None
                                for s0, pt0 in stash:
                                    nc.tensor.matmul(
                                        pv0[:], V_aug[:, s0, 2 * p, :],
                                        pt0[:, 0:512],
                                        start=(s0 == 0), stop=False)
                                    nc.tensor.matmul(
                                        pv1[:], V_aug[:, s0, 2 * p + 1, :],
                                        pt0[:, 512:1024],
                                        start=(s0 == 0), stop=False)
                                stash = []
                                nc.tensor.matmul(
                                    pv0[:], V_aug[:, sc, 2 * p, :],
                                    p_t[:, 0:512],
                                    start=(sc == 0), stop=(sc == SC - 1))
                                nc.tensor.matmul(
                                    pv1[:], V_aug[:, sc, 2 * p + 1, :],
                                    p_t[:, 512:1024],
                                    start=(sc == 0), stop=(sc == SC - 1))
                                # sprinkle the previous t-block's 8 WO
                                # chunks across pairs 0-1, every other sc,
                                # to stay under the ACT rate per slot
                                if (pending_wo is not None and p <= 1
                                        and 2 <= sc <= 9 and (sc % 2) == 0):
                                    widx = p * 4 + (sc - 2) // 2
                                    emit_wo_chunk(pending_wo[0], pending_wo[1],
                                                  widx // 2, widx % 2)
                                    if widx == 7:
                                        pending_wo = None
                            ctx_p = ctxpool.tile([P, 512], fp16, tag=f"ctx{p}",
                                                 name=f"ctx_{tb}_{p}")
                            pending = (tb, p, pv0, pv1, ctx_p)
                            ctx_tb.append(ctx_p)
                        pending_wo = (tb, ctx_tb)

                    # tail: last pair's evac + last t-block's WO
                    if pending is not None:
                        flush_evac(pending)
                        pending = None
                    if pending_wo is not None:
                        for ti in range(4):
                            for ob in range(2):
                                emit_wo_chunk(pending_wo[0], pending_wo[1],
                                              ti, ob)
                        pending_wo = None
    _nc_cache[0] = nc
    return nc


# ---------------------------------------------------------------- host side
def make_in_maps(keys, queries, values, WK_w, WK_b, WQ_w, WQ_b, WV_w, WV_b,
                 WO_w):
    keys = np.asarray(keys, dtype=np.float32)
    queries = np.asarray(queries, dtype=np.float32)
    values = np.asarray(values, dtype=np.float32)

    wblob = np.empty((2176, 2048), np.float16)
    wblob[WQK_OFF:WQK_OFF + 1024, 0:1024] = np.asarray(WQ_w, np.float32).T
    wblob[WQK_OFF:WQK_OFF + 1024, 1024:2048] = np.asarray(WK_w, np.float32).T
    wblob[WVO_OFF:WVO_OFF + 1024, 0:1024] = np.asarray(WV_w, np.float32).T
    wblob[WVO_OFF:WVO_OFF + 1024, 1024:2048] = np.asarray(WO_w, np.float32).T
    wblob[BIAS_OFF:BIAS_OFF + P, :] = 0
    wblob[BIAS_OFF:BIAS_OFF + P, 0:8] = \
        np.asarray(WQ_b, np.float32).reshape(8, P).T
    wblob[BIAS_OFF:BIAS_OFF + P, 8:16] = \
        np.asarray(WK_b, np.float32).reshape(8, P).T

    xblob = np.empty((B * XBLOB_CORE_ROWS, 2048), np.float16)
    for b in range(B):
        r = b * XBLOB_CORE_ROWS
        xblob[r + XQ_OFF:r + XQ_OFF + 1024] = queries[b].T
        xblob[r + XK_OFF:r + XK_OFF + 1024] = keys[b].T
        xblob[r + XV_OFF:r + XV_OFF + 1024] = values[b].T
    return [{"xblob": xblob, "wblob": wblob} for _ in range(N_CORES)]


def kernel(keys, queries, values, pad_mask, WK_w, WK_b, WQ_w, WQ_b, WV_w, WV_b,
           WO_w, WO_b):
    nc = build_nc()
    in_maps = make_in_maps(keys, queries, values, WK_w, WK_b, WQ_w, WQ_b,
                           WV_w, WV_b, WO_w)
    res = run_bass_kernel_spmd(nc, in_maps, list(range(N_CORES)))
    # free-dim biases folded on host: WO_b directly; WV_b exactly via
    # WV_b @ WO_w^T  (attention rows sum to 1).
    bias = (np.asarray(WO_b, np.float64)
            + np.asarray(WV_b, np.float64) @ np.asarray(WO_w, np.float64).T)
    out = np.empty((B, T, D), np.float32)
    for b in range(B):
        out[b] = (res.results[2 * b]["out"].astype(np.float64)
                  + res.results[2 * b + 1]["out"].astype(np.float64)
                  + bias).astype(np.float32)
    return out


# revision 21
# speedup vs baseline: 57.9384x; 1.1310x over previous
"""Bass/Trainium2 kernel for nn_CasualSelfAttention (B=4, T=2048, D=1024, H=16, dk=64).

Sharding: batch (4) x head-group (2) = 8 cores. Each core computes 8 heads of one
batch element end-to-end (QKV projections, attention, WO partial product); the
host sums the two head-group partials per batch and folds the free-dim biases.

Inputs are packed into two fp16 blobs whose content is IDENTICAL on every core
(so a host harness can pass them replicated / cached across calls); each core
selects its batch via a partition-id-dependent dynamic DMA offset
(b = pid//2 rows into xblob) and its head-group via a column/row shift
(g = pid%2) on the weight slices:
  xblob [12288, 2048]: batch b at rows b*3072: 0:1024 xq^T, 1024:2048 xk^T,
        2048:3072 xv^T
  wblob [2176, 2048]: rows 0:1024 = [wq^T | wk^T], rows 1024:2048 =
        [wv^T | wo^T], rows 2048:2176 cols 0:16 = [bq | bk]  (8 pairs each)

All big matmuls run on fp16 operands (fp32 PSUM accumulation -> only operand
quantization error, ~2^-11). Softmax sums use an augmented-V ones column
(M=65 PV matmuls); normalization uses a DVE reciprocal plus a K=1 PE
broadcast matmul.
"""
import sys
import os

sys.path.insert(0, '/opt/trn_rl_repo')

import numpy as np
import orjson

import concourse.bass as bass
import concourse.tile as tile
import concourse.mybir as mybir
from concourse.bass_utils import run_bass_kernel_spmd
from concourse.ap import AP

# ---------------------------------------------------------------- waitsplit
# The walrus build in this container accepts at most ONE semaphore wait per
# engine instruction.  Tile emits multi-wait sync_info; split the extras into
# single-wait NoOps on the same engine stream (in-order => semantically equal).
_ws_counter = [0]


_SELF_WAIT_ENGINES = ("Activation", "DVE")


def _split_instruction_waits(inst, out_list):
    si = inst.get("sync_info")
    if not si or not si.get("on_wait"):
        out_list.append(inst)
        return
    waits = si["on_wait"]
    # ACT/DVE execute strictly in order, so a compute instruction's wait on
    # its OWN engine's semaphore (slot-reuse WAW vs an older instruction on
    # the same engine) is always already satisfied — drop it instead of
    # spending a NoOp dispatch on the bottleneck ACT stream.
    eng = inst.get("engine")
    if (eng in _SELF_WAIT_ENGINES
            and inst.get("opcode") not in ("Drain", "EventSemaphore", "NoOp")):
        kept = [w for w in waits
                if w.get("ant_name", "").rsplit("_", 1)[0] != eng]
        if kept != waits:
            si = dict(si)
            si["on_wait"] = kept
            inst = dict(inst)
            inst["sync_info"] = si
            waits = kept
    if len(waits) <= 1:
        out_list.append(inst)
        return
    for w in waits[:-1]:
        _ws_counter[0] += 1
        out_list.append({
            "debug": inst.get("debug", 0),
            "engine": inst.get("engine"),
            "ins": [],
            "name": f"I-wsplit-{_ws_counter[0]}",
            "opcode": "NoOp",
            "outs": [],
            "sync_info": {"on_update": [], "on_wait": [w]},
        })
    si = dict(si)
    si["on_wait"] = [waits[-1]]
    inst = dict(inst)
    inst["sync_info"] = si
    out_list.append(inst)


def fix_multiwait_json(bir_bytes):
    d = orjson.loads(bir_bytes)
    for fn in d["functions"]:
        for bb in fn["blocks"]:
            new = []
            for inst in bb["instructions"]:
                _split_instruction_waits(inst, new)
            bb["instructions"] = new
    return orjson.dumps(d)


class WaitSplitBass(bass.Bass):
    def to_json_bytes(self):
        return fix_multiwait_json(super().to_json_bytes())


# ---------------------------------------------------------------- kernel build
P = 128
B, T, D = 4, 2048, 1024
N_CORES = 8           # batch (4) x head-group (2)
NH_LOC = 8            # heads per core
NP = NH_LOC // 2      # head pairs per core = 4
DK = 64
DC = D // P           # 8 d_model chunks
SC = T // P           # 16 s-chunks
NTB = T // 512        # 4 t-blocks
f32 = mybir.dt.float32
f32r = mybir.dt.float32r
fp16 = mybir.dt.float16
AF = mybir.ActivationFunctionType
MULT = mybir.AluOpType.mult

# xblob row offsets (within a batch's 3072-row block)
XQ_OFF, XK_OFF, XV_OFF = 0, 1024, 2048
XBLOB_CORE_ROWS = 3072
XBLOB_CORE_ELEMS = XBLOB_CORE_ROWS * 2048
# wblob row offsets
WQK_OFF = 0       # cols 0:1024 wq^T, 1024:2048 wk^T
WVO_OFF = 1024    # cols 0:1024 wv^T, 1024:2048 wo^T
BIAS_OFF = 2048   # rows 2048:2176, cols 0:8 bq (8 pairs), 8:16 bk

_nc_cache = [None]


def build_nc():
    if _nc_cache[0] is not None:
        return _nc_cache[0]
    nc = WaitSplitBass()
    xblob = nc.dram_tensor("xblob", [B * XBLOB_CORE_ROWS, 2048], fp16,
                           kind="ExternalInput")
    wblob = nc.dram_tensor("wblob", [2176, 2048], fp16, kind="ExternalInput")
    out = nc.dram_tensor("out", [T, D], fp16, kind="ExternalOutput")

    with tile.TileContext(nc) as tc:
        # every core receives identical blobs; batch = pid//2, head-group =
        # pid%2 select this core's slices via dynamic AP offsets
        pid = nc.partition_id()
        b_off = (pid // 2) * XBLOB_CORE_ELEMS
        g = pid % 2
        g_col = g * 512           # column shift for wq/wk/wv slices
        g_row = g * (512 * 2048)  # row shift for wo slice
        g_b = g * 4               # bias column shift

        def dyn(s, off):
            return AP(tensor=s.tensor, offset=s.offset + off, ap=s.ap)

        def xsrc(xoff, tb):
            # [P, DC, 512] view of this core's x tensor block, t-block tb
            s = xblob[xoff:xoff + 1024, tb * 512:(tb + 1) * 512] \
                .rearrange("(c p) t -> p c t", p=P)
            return dyn(s, b_off)

        with tc.tile_pool(name="persist", bufs=1) as persist, \
             tc.tile_pool(name="psProj", bufs=2, space="PSUM") as psProj, \
             tc.tile_pool(name="psS", bufs=2, space="PSUM") as psS, \
             tc.tile_pool(name="psPV", bufs=1, space="PSUM") as psPV:

            # ---- persistent tiles ----
            qT2 = [persist.tile([P, T], fp16, tag=f"qT2_{p}", name=f"qT2_{p}")
                   for p in range(NP)]
            kT2 = [persist.tile([P, T], fp16, tag=f"kT2_{p}", name=f"kT2_{p}")
                   for p in range(NP)]
            V_aug = persist.tile([P, SC, NH_LOC, 65], fp16, name="V_aug")
            nc.vector.memset(V_aug[:, :, :, 64], 1.0)
            b16 = persist.tile([P, 8], fp16, name="b16")
            nc.sync.dma_start(b16[:, 0:4],
                              dyn(wblob[BIAS_OFF:BIAS_OFF + P, 0:4], g_b))
            nc.sync.dma_start(b16[:, 4:8],
                              dyn(wblob[BIAS_OFF:BIAS_OFF + P, 8:12], g_b))
            bqk = persist.tile([P, 8], f32, name="bqk")
            nc.vector.tensor_copy(bqk[:], b16[:])
            ones64 = persist.tile([1, 64], f32r, name="ones64")
            nc.vector.memset(ones64[:].bitcast(f32), 1.0)

            # ---- phase A: projections (weights + X^T streamed per chunk) ----
            # 4 concurrent psum groups: 2 slots borrowed from the (idle)
            # scores pool + 2 from psProj.
            def alloc4(stem):
                ps = [psS.tile([P, 1024], f32, tag="scores",
                               name=f"{stem}_s{j}")[:, 0:512] for j in range(2)]
                ps += [psProj.tile([P, 512], f32, tag="proj",
                                   name=f"{stem}_p{j}") for j in range(2)]
                return ps

            ctx_00 = persist.tile([P, 512], fp16, name="ctx_00")
            # stash for (t-block 0, pairs 1-2) exp results computed during
            # phase A (scores+exp run early; PV waits until the PV psum
            # banks free up after pair 0's evacuation)
            stash_pt = {1: [None] * SC, 2: [None] * SC}
            with tc.tile_pool(name="spool", bufs=2 * SC) as spool, \
                 tc.tile_pool(name="pearly", bufs=2) as pearly, \
                 tc.tile_pool(name="wpool", bufs=1) as wpool, \
                 tc.tile_pool(name="xpool", bufs=2) as xpool:
                wq = wpool.tile([P, DC, 512], fp16, tag="wq", name="wq")
                wk = wpool.tile([P, DC, 512], fp16, tag="wk", name="wk")
                wv = wpool.tile([P, DC, 512], fp16, tag="wv", name="wv")

                def wsrc(row0, col0):
                    s = wblob[row0:row0 + 1024, col0:col0 + 512] \
                        .rearrange("(c p) t -> p c t", p=P)
                    return dyn(s, g_col)

                nc.sync.dma_start(wq[:], wsrc(WQK_OFF, 0))
                nc.sync.dma_start(wk[:], wsrc(WQK_OFF, 1024))
                nc.sync.dma_start(wv[:], wsrc(WVO_OFF, 0))

                for tb in range(NTB):
                    ts_ = slice(tb * 512, (tb + 1) * 512)
                    # xq+xk+xv rows are adjacent in xblob -> ONE dynamic DMA
                    # per t-block (dynamic DMAs each pin a queue register
                    # pair; the pool is small, so batch aggressively).
                    x3_all = xpool.tile([P, 3 * DC, 512], fp16, tag="x3",
                                        name=f"x3_{tb}")
                    s_x = xblob[0:3 * 1024, tb * 512:(tb + 1) * 512] \
                        .rearrange("(c p) t -> p c t", p=P)
                    nc.sync.dma_start(x3_all[:], dyn(s_x, b_off))
                    # q then k: 4 pair-groups, chunk-outer accumulation
                    for qk, (w_t, boff, dst) in enumerate(
                            ((wq, 0, qT2), (wk, 4, kT2))):
                        x_all = x3_all[:, qk * DC:(qk + 1) * DC, :]
                        ps4 = alloc4(f"psqk{tb}_{qk}")
                        for c in range(DC):
                            for p in range(NP):
                                nc.tensor.matmul(
                                    ps4[p][:], w_t[:, c, p * P:(p + 1) * P],
                                    x_all[:, c, :], start=(c == 0),
                                    stop=(c == DC - 1))
                        for p in range(NP):
                            nc.vector.tensor_scalar_add(
                                dst[p][:, ts_], ps4[p][:],
                                bqk[:, boff + p:boff + p + 1])
                    # v: 4 t-tile groups, chunk-outer
                    xv_all = x3_all[:, 2 * DC:3 * DC, :]
                    ps4 = alloc4(f"psv{tb}")
                    for c in range(DC):
                        for ti in range(4):
                            nc.tensor.matmul(
                                ps4[ti][:], xv_all[:, c, ti * P:(ti + 1) * P],
                                wv[:, c], start=(c == 0), stop=(c == DC - 1))
                    for ti in range(4):
                        tt = tb * 4 + ti
                        nc.vector.tensor_copy(
                            V_aug[:, tt, :, 0:64],
                            ps4[ti][:].rearrange("p (h d) -> p h d", d=64))

                    # early attention for (t-block 0, pair 0): its PV psum is
                    # idle during phase A and every dep of s-chunk quarter tb
                    # is produced by A(tb) — run it here so ACT starts early
                    # instead of idling through all projections.
                    if tb == 0:
                        pv_e0 = psPV.tile([65, 512], f32, tag="pv0",
                                          name="pv0_0_0")
                        pv_e1 = psPV.tile([65, 512], f32, tag="pv1",
                                          name="pv1_0_0")
                    for sc in range(4 * tb, 4 * tb + 4):
                        ss = slice(sc * P, (sc + 1) * P)
                        ps_s = psS.tile([P, 1024], f32, tag="scores",
                                        name=f"ps_s_0_0_{sc}")
                        nc.tensor.matmul(
                            ps_s[:, 0:512], qT2[0][0:64, ss],
                            kT2[0][0:64, 0:512], start=True, stop=True,
                            tile_position=(0, 0))
                        nc.tensor.matmul(
                            ps_s[:, 512:1024], qT2[0][64:128, ss],
                            kT2[0][64:128, 0:512], start=True, stop=True,
                            tile_position=(64, 0))
                        p_t = pearly.tile([P, 1024], fp16, tag="pe",
                                          name=f"pe_{sc}")
                        nc.scalar.activation(p_t[:], ps_s[:], AF.Exp,
                                             scale=0.125)
                        nc.tensor.matmul(
                            pv_e0[:], V_aug[:, sc, 0, :], p_t[:, 0:512],
                            start=(sc == 0), stop=(sc == SC - 1))
                        nc.tensor.matmul(
                            pv_e1[:], V_aug[:, sc, 1, :], p_t[:, 512:1024],
                            start=(sc == 0), stop=(sc == SC - 1))

            # ---- phases B+C per t-block ----
            with tc.tile_pool(name="ppool", bufs=5) as ppool, \
                 tc.tile_pool(name="rbpool", bufs=2) as rbpool, \
                 tc.tile_pool(name="ctxpool", bufs=2) as ctxpool, \
                 tc.tile_pool(name="wopool", bufs=1) as wopool, \
                 tc.tile_pool(name="opool", bufs=3) as opool:
                    wo = wopool.tile([P, NP, D], fp16, name="wo")
                    s_wo = wblob[WVO_OFF:WVO_OFF + NP * P, 1024:2048] \
                        .rearrange("(c p) t -> p c t", p=P)
                    nc.sync.dma_start(wo[:], dyn(s_wo, g_row))

                    def flush_evac(pend):
                        # normalize pair into its ctx tile:
                        # ctx[h] = pv[h][0:64] * bcast(1 / pv[h][64])
                        tb, p, pv0, pv1, ctx_p = pend
                        for h, pv in ((0, pv0), (1, pv1)):
                            r_t = rbpool.tile([1, 512], f32r, tag="r",
                                              name=f"r_{tb}_{p}_{h}")
                            with nc.allow_low_precision(reason="softmax recip"):
                                nc.vector.reciprocal(r_t[:], pv[64:65, :])
                            ps_rb = psProj.tile([64, 512], f32, tag="proj",
                                                name=f"ps_rb_{tb}_{p}_{h}")
                            nc.tensor.matmul(ps_rb[:], ones64[:], r_t[:],
                                             start=True, stop=True)
                            rb_s = rbpool.tile([64, 512], f32, tag="rb",
                                               name=f"rb_{tb}_{p}_{h}")
                            nc.vector.tensor_copy(rb_s[:], ps_rb[:])
                            nc.vector.tensor_tensor(
                                ctx_p[h * 64:(h + 1) * 64, :],
                                pv[0:64, :], rb_s[:], MULT)

                    def emit_wo_chunk(wtb, wctx, ti, ob):
                        # one [128t, 512o] WO output tile of t-block wtb
                        ps_o = psProj.tile([P, 512], f32, tag="proj",
                                           name=f"ps_o_{wtb}_{ti}_{ob}")
                        for p in range(NP):
                            nc.tensor.matmul(
                                ps_o[:], wctx[p][:, ti * P:(ti + 1) * P],
                                wo[:, p, ob * 512:(ob + 1) * 512],
                                start=(p == 0), stop=(p == NP - 1))
                        o_t = opool.tile([P, 512], fp16, tag="o",
                                         name=f"o_{wtb}_{ti}_{ob}")
                        nc.vector.tensor_copy(o_t[:], ps_o[:])
                        nc.sync.dma_start(
                            out[wtb * 512 + ti * P: wtb * 512 + (ti + 1) * P,
                                ob * 512:(ob + 1) * 512], o_t[:])

                    # (t-block 0, pair 0) already ran during phase A; seed its
                    # deferred evacuation so pair 1's sc==2 flush handles it.
                    pending = (0, 0, pv_e0, pv_e1, ctx_00)
                    pending_wo = None     # (tb, ctx_tb) whose WO is deferred
                    for tb in range(NTB):
                        ts_ = slice(tb * 512, (tb + 1) * 512)
                        ctx_tb = [ctx_00] if tb == 0 else []
                        for p in range(NP):
                            if tb == 0 and p == 0:
                                continue
                            # -- B: attention for (pair p, t-block tb) --
                            pv0 = psPV.tile([65, 512], f32, tag="pv0",
                                            name=f"pv0_{tb}_{p}")
                            pv1 = psPV.tile([65, 512], f32, tag="pv1",
                                            name=f"pv1_{tb}_{p}")
                            # Defer the previous pair's PV-psum evacuation (and
                            # the previous t-block's WO chunks) past this
                            # pair's first score/exp groups so ACT stays fed
                            # while PE runs the evac/WO work in its slack.
                            stash = []
                            for sc in range(SC):
                                ss = slice(sc * P, (sc + 1) * P)
                                ps_s = psS.tile([P, 1024], f32, tag="scores",
                                                name=f"ps_s_{tb}_{p}_{sc}")
                                nc.tensor.matmul(
                                    ps_s[:, 0:512], qT2[p][0:64, ss],
                                    kT2[p][0:64, ts_], start=True, stop=True,
                                    tile_position=(0, 0))
                                nc.tensor.matmul(
                                    ps_s[:, 512:1024], qT2[p][64:128, ss],
                                    kT2[p][64:128, ts_], start=True, stop=True,
                                    tile_position=(64, 0))
                                p_t = ppool.tile([P, 1024], fp16, tag="p",
                                                 name=f"p_{tb}_{p}_{sc}")
                                nc.scalar.activation(p_t[:], ps_s[:], AF.Exp,
                                                     scale=0.125)
                                if pending is not None and sc < 2:
                                    stash.append((sc, p_t))
                                    continue
                                if pending is not None and sc == 2:
                                    flush_evac(pending)
                                    pending = None
                                for s0, pt0 in stash:
                                    nc.tensor.matmul(
                                        pv0[:], V_aug[:, s0, 2 * p, :],
                                        pt0[:, 0:512],
                                        start=(s0 == 0), stop=False)
                                    nc.tensor.matmul(
                                        pv1[:], V_aug[:, s0, 2 * p + 1, :],
                                        pt0[:, 512:1024],
                                        start=(s0 == 0), stop=False)
                                stash = []
                                nc.tensor.matmul(
                                    pv0[:], V_aug[:, sc, 2 * p, :],
                                    p_t[:, 0:512],
                                    start=(sc == 0), stop=(sc == SC - 1))
                                nc.tensor.matmul(
                                    pv1[:], V_aug[:, sc, 2 * p + 1, :],
                                    p_t[:, 512:1024],
                                    start=(sc == 0), stop=(sc == SC - 1))
                                # sprinkle the previous t-block's 8 WO
                                # chunks across pairs 0-1, every other sc,
                                # to stay under the ACT rate per slot
                                if (pending_wo is not None and p <= 1
                                        and 2 <= sc <= 9 and (sc % 2) == 0):
                                    widx = p * 4 + (sc - 2) // 2
                                    emit_wo_chunk(pending_wo[0], pending_wo[1],
                                                  widx // 2, widx % 2)
                                    if widx == 7:
                                        pending_wo = None
                            ctx_p = ctxpool.tile([P, 512], fp16, tag=f"ctx{p}",
                                                 name=f"ctx_{tb}_{p}")
                            pending = (tb, p, pv0, pv1, ctx_p)
                            ctx_tb.append(ctx_p)
                        pending_wo = (tb, ctx_tb)

                    # tail: last pair's evac + last t-block's WO
                    if pending is not None:
                        flush_evac(pending)
                        pending = None
                    if pending_wo is not None:
                        for ti in range(4):
                            for ob in range(2):
                                emit_wo_chunk(pending_wo[0], pending_wo[1],
                                              ti, ob)
                        pending_wo = None
    _nc_cache[0] = nc
    return nc


# ---------------------------------------------------------------- host side
def make_in_maps(keys, queries, values, WK_w, WK_b, WQ_w, WQ_b, WV_w, WV_b,
                 WO_w):
    keys = np.asarray(keys, dtype=np.float32)
    queries = np.asarray(queries, dtype=np.float32)
    values = np.asarray(values, dtype=np.float32)

    wblob = np.empty((2176, 2048), np.float16)
    wblob[WQK_OFF:WQK_OFF + 1024, 0:1024] = np.asarray(WQ_w, np.float32).T
    wblob[WQK_OFF:WQK_OFF + 1024, 1024:2048] = np.asarray(WK_w, np.float32).T
    wblob[WVO_OFF:WVO_OFF + 1024, 0:1024] = np.asarray(WV_w, np.float32).T
    wblob[WVO_OFF:WVO_OFF + 1024, 1024:2048] = np.asarray(WO_w, np.float32).T
    wblob[BIAS_OFF:BIAS_OFF + P, :] = 0
    wblob[BIAS_OFF:BIAS_OFF + P, 0:8] = \
        np.asarray(WQ_b, np.float32).reshape(8, P).T
    wblob[BIAS_OFF:BIAS_OFF + P, 8:16] = \
        np.asarray(WK_b, np.float32).reshape(8, P).T

    xblob = np.empty((B * XBLOB_CORE_ROWS, 2048), np.float16)
    for b in range(B):
        r = b * XBLOB_CORE_ROWS
        xblob[r + XQ_OFF:r + XQ_OFF + 1024] = queries[b].T
        xblob[r + XK_OFF:r + XK_OFF + 1024] = keys[b].T
        xblob[r + XV_OFF:r + XV_OFF + 1024] = values[b].T
    return [{"xblob": xblob, "wblob": wblob} for _ in range(N_CORES)]


def kernel(keys, queries, values, pad_mask, WK_w, WK_b, WQ_w, WQ_b, WV_w, WV_b,
           WO_w, WO_b):
    nc = build_nc()
    in_maps = make_in_maps(keys, queries, values, WK_w, WK_b, WQ_w, WQ_b,
                           WV_w, WV_b, WO_w)
    res = run_bass_kernel_spmd(nc, in_maps, list(range(N_CORES)))
    # free-dim biases folded on host: WO_b directly; WV_b exactly via
    # WV_b @ WO_w^T  (attention rows sum to 1).
    bias = (np.asarray(WO_b, np.float64)
            + np.asarray(WV_b, np.float64) @ np.asarray(WO_w, np.float64).T)
    out = np.empty((B, T, D), np.float32)
    for b in range(B):
        out[b] = (res.results[2 * b]["out"].astype(np.float64)
                  + res.results[2 * b + 1]["out"].astype(np.float64)
                  + bias).astype(np.float32)
    return out
